# revision 1
# baseline (speedup 1.0000x reference)
"""Trainium2 Bass kernel for ConstraintViolationLoss (GNN message passing).

Strategy (8 NeuronCores, SPMD):
  - Host does index-only layout prep: sort edges by constraint, classify each
    constraint into a degree tier (stride 16/24/32/40/48/96...), assign every
    constraint to one of 1024 (core, partition) bins, and pad each
    constraint's edge list to its tier stride so the per-constraint
    segment-sum becomes a fixed-stride reduction.
  - Launch 1 (8 cores, sharded over the small-int vars): softmax
    expected-value head: expected = softmax(logits) @ [0..C) + offsets.
  - Host assembles the per-edge-slot gathered value stream xg (pure indexed
    copies of input values / launch-1 output; no arithmetic).
  - Launch 2 (8 cores, constraints sharded into bins): w = xg * feat,
    per-segment strided reduce -> Ax, violation = relu(Ax - bias), then
    per-partition sum / max / count partials.
  - Host combines 1024 partial rows into the 4 scalar outputs.
"""

import sys

sys.path.insert(0, "/opt/trn_rl_repo")

import numpy as np

import concourse.bass as bass
import concourse.mybir as mybir
from concourse.bass_utils import run_bass_kernel_spmd

P = 128
NCORES = 8
NBINS = P * NCORES
LAMBDA_MEAN, LAMBDA_MAX = 1.0, 0.1
BIAS_COL = 1
LP_SOL_COL = 8
TIER_LS = [16, 24, 32, 40, 48]   # degree-class strides; overflow tier appended
CHUNK_ELEMS = 6144               # target elems/partition per pipelined chunk
BIG_BIAS = 1.0e30
F32 = mybir.dt.float32

# shapes of the most recent build, for test harness introspection
LAST_ROWS_PP = None
LAST_P2_ARGS = None
LA = 48  # legacy alias used by older validation scripts


def _build_phase1(rows_pp: int, nchunks: int = 4, ccls: int = 16):
    """expected = softmax(logits) @ cls + offsets; rows_pp rows/partition."""
    global LAST_ROWS_PP
    LAST_ROWS_PP = rows_pp
    assert rows_pp % nchunks == 0
    rc = rows_pp // nchunks
    fc = rc * ccls
    nf = rows_pp * ccls
    nc = bass.Bass()
    lg = nc.declare_dram_parameter("logits", [P, nf], F32, isOutput=False)
    cp = nc.declare_dram_parameter("clspat", [P, fc], F32, isOutput=False)
    of = nc.declare_dram_parameter("offs", [P, rows_pp], F32, isOutput=False)
    ex = nc.declare_dram_parameter("expected", [P, rows_pp], F32, isOutput=True)

    with (
        nc.sbuf_tensor([P, 2, fc], F32) as tl,
        nc.sbuf_tensor([P, 2, rc], F32) as tof,
        nc.sbuf_tensor([P, fc], F32) as tcp,
        nc.sbuf_tensor([P, fc], F32) as te,
        nc.sbuf_tensor([P, rc], F32) as tden,
        nc.sbuf_tensor([P, rc], F32) as tnum,
        nc.sbuf_tensor([P, rows_pp], F32) as tout,
        nc.Block() as block,
        nc.semaphore("pl0") as pl0,
        nc.semaphore("pl1") as pl1,
        nc.semaphore("csem") as csem,
        nc.semaphore("ssem") as ssem,
        nc.semaphore("vsem") as vsem,
        nc.semaphore("osem") as osem,
    ):
        pl = [pl0, pl1]

        @block.sync
        def _(sync):
            sync.dma_start(out=tcp[:], in_=cp[:]).then_inc(csem, 16)
            for c in range(nchunks):
                if c >= 2:
                    sync.wait_ge(vsem, c - 1)
                b = c % 2
                sync.dma_start(
                    out=tl[:, b, :], in_=lg[:, c * fc : (c + 1) * fc]
                ).then_inc(pl[b], 16)
                sync.dma_start(
                    out=tof[:, b, :], in_=of[:, c * rc : (c + 1) * rc]
                ).then_inc(pl[b], 16)
            sync.wait_ge(vsem, nchunks)
            sync.dma_start(out=ex[:], in_=tout[:]).then_inc(osem, 16)
            sync.wait_ge(osem, 16)

        @block.scalar
        def _(scalar):
            for c in range(nchunks):
                b = c % 2
                scalar.wait_ge(pl[b], 32 * (c // 2 + 1))
                nc.scalar.activation(
                    out=tl[:, b, :], in_=tl[:, b, :],
                    func=mybir.ActivationFunctionType.Exp,
                ).then_inc(ssem, 1)

        @block.vector
        def _(vector):
            vector.wait_ge(csem, 16)
            for c in range(nchunks):
                b = c % 2
                vector.wait_ge(ssem, c + 1)
                nc.vector.drain()
                g = tl[:, b, :].rearrange("p (r c) -> p r c", c=ccls)
                nc.vector.tensor_reduce(
                    out=tden[:], in_=g,
                    axis=mybir.AxisListType.X, op=mybir.AluOpType.add,
                )
                nc.vector.tensor_tensor(
                    out=te[:], in0=tl[:, b, :], in1=tcp[:],
                    op=mybir.AluOpType.mult,
                )
                nc.vector.drain()
                nc.vector.tensor_reduce(
                    out=tnum[:],
                    in_=te[:].rearrange("p (r c) -> p r c", c=ccls),
                    axis=mybir.AxisListType.X, op=mybir.AluOpType.add,
                )
                nc.vector.reciprocal(out=tden[:], in_=tden[:])
                nc.vector.drain()
                nc.vector.tensor_tensor(
                    out=tnum[:], in0=tnum[:], in1=tden[:],
                    op=mybir.AluOpType.mult,
                )
                nc.vector.drain()
                nc.vector.tensor_tensor(
                    out=tout[:, c * rc : (c + 1) * rc],
                    in0=tnum[:], in1=tof[:, b, :], op=mybir.AluOpType.add,
                )
                nc.vector.drain().then_inc(vsem, 1)

    return nc


def _build_phase2(tiers):
    """Per-core segment reduce + loss partials.

    tiers: list of (sa, L, ca) — segments/partition, stride, chunk segments.
    """
    global LAST_P2_ARGS
    LAST_P2_ARGS = (tiers,)
    nc = bass.Bass()
    xg, ft, bs = [], [], []
    for r, (sa, L, ca) in enumerate(tiers):
        xg.append(nc.declare_dram_parameter(f"xg{r}", [P, sa * L], F32, False))
        ft.append(nc.declare_dram_parameter(f"ft{r}", [P, sa * L], F32, False))
        bs.append(nc.declare_dram_parameter(f"bs{r}", [P, sa], F32, False))
    out_p = nc.declare_dram_parameter("partials", [P, 4], F32, isOutput=True)

    fmax = max(ca * L for sa, L, ca in tiers)
    cmax = max(ca for sa, L, ca in tiers)
    chunks = []  # (tier, chunk_idx)
    for r, (sa, L, ca) in enumerate(tiers):
        for i in range(sa // ca):
            chunks.append((r, i))

    with (
        nc.sbuf_tensor([P, 2, fmax], F32) as tx,
        nc.sbuf_tensor([P, 2, fmax], F32) as tf,
        nc.sbuf_tensor([P, 2, cmax], F32) as tb,
        nc.sbuf_tensor([P, cmax], F32) as tax,
        nc.sbuf_tensor([P, cmax], F32) as tviol,
        nc.sbuf_tensor([P, cmax], F32) as tgt,
        nc.sbuf_tensor([P, 1], F32) as ts,
        nc.sbuf_tensor([P, 1], F32) as ts2,
        nc.sbuf_tensor([P, 1], F32) as ts3,
        nc.sbuf_tensor([P, 1], F32) as asum,
        nc.sbuf_tensor([P, 1], F32) as amax,
        nc.sbuf_tensor([P, 1], F32) as acnt,
        nc.sbuf_tensor([P, 4], F32) as tout,
        nc.Block() as block,
        nc.semaphore("pa0") as pa0,
        nc.semaphore("pa1") as pa1,
        nc.semaphore("osem") as osem,
        nc.semaphore("vsem") as vsem,
    ):
        pa = [pa0, pa1]

        @block.sync
        def _(sync):
            for g, (r, i) in enumerate(chunks):
                sa, L, ca = tiers[r]
                fc = ca * L
                if g >= 2:
                    sync.wait_ge(vsem, g - 1)
                b = g % 2
                sync.dma_start(
                    out=tx[:, b, :fc], in_=xg[r][:, i * fc : (i + 1) * fc]
                ).then_inc(pa[b], 16)
                sync.dma_start(
                    out=tf[:, b, :fc], in_=ft[r][:, i * fc : (i + 1) * fc]
                ).then_inc(pa[b], 16)
                sync.dma_start(
                    out=tb[:, b, :ca], in_=bs[r][:, i * ca : (i + 1) * ca]
                ).then_inc(pa[b], 16)
            sync.wait_ge(vsem, len(chunks) + 1)
            sync.dma_start(out=out_p[:], in_=tout[:]).then_inc(osem, 16)
            sync.wait_ge(osem, 16)

        @block.vector
        def _(vector):
            nc.vector.memset(asum[:], 0.0)
            nc.vector.memset(amax[:], 0.0)
            nc.vector.memset(acnt[:], 0.0)

            def seg_chunk(xa, fa_, ba, nseg, ls):
                """Accumulate violation stats for nseg segments of stride ls."""
                nc.vector.drain()
                nc.vector.tensor_tensor(
                    out=xa, in0=xa, in1=fa_, op=mybir.AluOpType.mult
                )
                nc.vector.drain()
                nc.vector.tensor_reduce(
                    out=tax[:, :nseg],
                    in_=xa.rearrange("p (s l) -> p s l", l=ls),
                    axis=mybir.AxisListType.X, op=mybir.AluOpType.add,
                )
                nc.vector.drain()
                nc.vector.tensor_tensor(
                    out=tviol[:, :nseg], in0=tax[:, :nseg], in1=ba,
                    op=mybir.AluOpType.subtract,
                )
                nc.vector.drain()
                nc.vector.tensor_scalar_max(
                    out=tviol[:, :nseg], in0=tviol[:, :nseg], scalar1=0.0
                )
                nc.vector.drain()
                # the three reads of tviol are independent of each other
                nc.vector.tensor_reduce(
                    out=ts[:], in_=tviol[:, :nseg],
                    axis=mybir.AxisListType.X, op=mybir.AluOpType.add,
                )
                nc.vector.tensor_reduce(
                    out=ts2[:], in_=tviol[:, :nseg],
                    axis=mybir.AxisListType.X, op=mybir.AluOpType.max,
                )
                nc.vector.tensor_scalar(
                    out=tgt[:, :nseg], in0=tviol[:, :nseg],
                    scalar1=1e-6, scalar2=None, op0=mybir.AluOpType.is_gt,
                )
                nc.vector.drain()
                nc.vector.tensor_tensor(
                    out=asum[:], in0=asum[:], in1=ts[:], op=mybir.AluOpType.add
                )
                nc.vector.tensor_tensor(
                    out=amax[:], in0=amax[:], in1=ts2[:], op=mybir.AluOpType.max
                )
                nc.vector.tensor_reduce(
                    out=ts3[:], in_=tgt[:, :nseg],
                    axis=mybir.AxisListType.X, op=mybir.AluOpType.add,
                )
                nc.vector.drain()
                nc.vector.tensor_tensor(
                    out=acnt[:], in0=acnt[:], in1=ts3[:], op=mybir.AluOpType.add
                )

            for g, (r, i) in enumerate(chunks):
                sa, L, ca = tiers[r]
                fc = ca * L
                b = g % 2
                vector.wait_ge(pa[b], 48 * (g // 2 + 1))
                seg_chunk(tx[:, b, :fc], tf[:, b, :fc], tb[:, b, :ca], ca, L)
                nc.vector.drain().then_inc(vsem, 1)
            nc.vector.tensor_copy(out=tout[:, 0:1], in_=asum[:])
            nc.vector.tensor_copy(out=tout[:, 1:2], in_=amax[:])
            nc.vector.tensor_copy(out=tout[:, 2:3], in_=acnt[:])
            nc.vector.tensor_copy(out=tout[:, 3:4], in_=acnt[:])
            nc.vector.drain().then_inc(vsem, 1)

    return nc


def _round_up(x: int, m: int) -> int:
    return (x + m - 1) // m * m


def kernel(**inputs) -> tuple:
    prob_bin = np.asarray(inputs["prob_bin"], dtype=np.float32)
    logits = np.asarray(inputs["logits_int_small"], dtype=np.float32)
    offsets = np.asarray(inputs["int_small_offsets"], dtype=np.float32)
    pred_l = np.asarray(inputs["pred_int_large"], dtype=np.float32)
    feat = np.asarray(inputs["edge_features"], dtype=np.float32).reshape(-1)
    cfeat = np.asarray(inputs["constraint_features"], dtype=np.float32)
    vfeat = np.asarray(inputs["variable_features"], dtype=np.float32)
    idx_bin = np.asarray(inputs["idx_bin"], dtype=np.int64)
    idx_s = np.asarray(inputs["idx_int_small"], dtype=np.int64)
    idx_l = np.asarray(inputs["idx_int_large"], dtype=np.int64)
    var_types = np.asarray(inputs["var_types"], dtype=np.int64)
    ei = np.asarray(inputs["edge_indices"], dtype=np.int64)
    n_vars = int(inputs["n_vars"])

    n_con = cfeat.shape[0]
    ns, ccls = logits.shape
    bias = np.ascontiguousarray(cfeat[:, BIAS_COL])
    lp_vals = np.ascontiguousarray(vfeat[:, LP_SOL_COL])
    con = ei[0]
    var = ei[1]
    ne = con.shape[0]

    # ---------------- host index prep (layout only) ----------------
    deg = np.bincount(con, minlength=n_con)
    order = np.argsort(con, kind="stable")
    run_start = np.zeros(n_con + 1, dtype=np.int64)
    np.cumsum(deg, out=run_start[1:])
    off_in_run = np.arange(ne, dtype=np.int64) - run_start[con[order]]
    con_sorted = con[order]
    var_sorted = var[order].astype(np.int32)
    feat_sorted = feat[order]

    max_deg = int(deg.max()) if ne else 0
    strides = list(TIER_LS)
    if max_deg > strides[-1]:
        strides.append(max(96, _round_up(max_deg, 16)))
    # tier id per constraint: first stride >= deg
    tier_of = np.searchsorted(np.asarray(strides), deg, side="left")

    tiers = []        # (sa, L, ca) per tier with any segments
    tier_remap = {}   # original stride index -> dense tier index
    bin_of = np.zeros(n_con, dtype=np.int64)
    rank_of = np.zeros(n_con, dtype=np.int64)
    for si, L in enumerate(strides):
        cons = np.nonzero(tier_of == si)[0]
        if cons.size == 0:
            continue
        rank_order = cons[np.argsort(-deg[cons], kind="stable")]
        ar = np.arange(rank_order.size, dtype=np.int64)
        bin_of[rank_order] = ar % NBINS
        rank_of[rank_order] = ar // NBINS
        sa_need = max(int((rank_order.size + NBINS - 1) // NBINS), 1)
        n_chunks = max(1, -(-sa_need * L // CHUNK_ELEMS))
        ca = -(-sa_need // n_chunks)
        sa = ca * n_chunks
        tier_remap[si] = len(tiers)
        tiers.append((sa, L, ca))

    # per-edge destination slots, per tier
    e_tier = tier_of[con_sorted]
    xgv, ftv, bsv = [], [], []
    for si, r in sorted(tier_remap.items()):
        sa, L, ca = tiers[r]
        sel = e_tier == si
        cs = con_sorted[sel]
        idx = (bin_of[cs] * sa + rank_of[cs]) * L + off_in_run[sel]
        ftr = np.zeros(NBINS * sa * L, dtype=np.float32)
        varr = np.zeros(NBINS * sa * L, dtype=np.int32)
        ftr[idx] = feat_sorted[sel]
        varr[idx] = var_sorted[sel]
        cons = np.nonzero(tier_of == si)[0]
        bsr = np.full(NBINS * sa, BIG_BIAS, dtype=np.float32)
        bsr[bin_of[cons] * sa + rank_of[cons]] = bias[cons]
        ftv.append(ftr)
        xgv.append(varr)
        bsv.append(bsr)

    # ---------------- launch 1: expected values ----------------
    nch1 = 4
    rows_pp = _round_up((ns + NCORES * P - 1) // (NCORES * P), nch1)
    ns_pad = NCORES * P * rows_pp
    lg_pad = np.zeros((ns_pad, ccls), dtype=np.float32)
    lg_pad[:ns] = logits
    of_pad = np.zeros(ns_pad, dtype=np.float32)
    of_pad[:ns] = offsets
    rc = rows_pp // nch1
    clspat = np.tile(np.arange(ccls, dtype=np.float32), rc)[None].repeat(P, 0)

    nc1 = _build_phase1(rows_pp, nch1, ccls)
    lg_r = lg_pad.reshape(NCORES, P, rows_pp * ccls)
    of_r = of_pad.reshape(NCORES, P, rows_pp)
    in1 = [
        {"logits": lg_r[c], "clspat": clspat, "offs": of_r[c]} for c in range(NCORES)
    ]
    res1 = run_bass_kernel_spmd(nc1, in1, list(range(NCORES)))
    expected = np.concatenate(
        [res1.results[c]["expected"].reshape(-1) for c in range(NCORES)]
    )[:ns]

    # ---------------- host: assemble x and gather streams ----------------
    xfull = np.zeros(n_vars, dtype=np.float32)
    xfull[idx_bin] = prob_bin[:, 0]
    xfull[idx_s] = expected
    xfull[idx_l] = pred_l[:, 0]
    xfull = np.where(var_types == 0, lp_vals, xfull)

    # ---------------- launch 2: segment reduce + loss partials ----------------
    nc2 = _build_phase2(tiers)
    in2 = []
    for c in range(NCORES):
        m = {}
        for r, (sa, L, ca) in enumerate(tiers):
            m[f"xg{r}"] = xfull[xgv[r].reshape(NCORES, P, sa * L)[c]]
            m[f"ft{r}"] = ftv[r].reshape(NCORES, P, sa * L)[c]
            m[f"bs{r}"] = bsv[r].reshape(NCORES, P, sa)[c]
        in2.append(m)
    res2 = run_bass_kernel_spmd(nc2, in2, list(range(NCORES)))

    parts = np.stack([res2.results[c]["partials"] for c in range(NCORES)])
    vsum = np.float32(parts[:, :, 0].astype(np.float64).sum())
    vmax = np.float32(parts[:, :, 1].max())
    vcnt = np.int32(round(float(parts[:, :, 2].sum())))
    mean_viol = np.float32(vsum / np.float32(n_con))
    penalty = np.float32(
        np.float32(LAMBDA_MEAN) * mean_viol + np.float32(LAMBDA_MAX) * vmax
    )
    return penalty, mean_viol, vmax, vcnt



# revision 16
# speedup vs baseline: 1.2454x; 1.2454x over previous
"""Trainium2 Bass kernel for ConstraintViolationLoss (GNN message passing).

Two launches on 8 NeuronCores (SPMD), fp16 data streams:

  Launch 1 (softmax expected-value head): logits are laid out class-on-
  partition ([128, ncol] tiles, partition p = 16*g + c holding class c of
  row-group g), ACT computes exp in fp16, and ONE PE matmul against a
  constant [128, 16] weight block produces both softmax sums per row
  (denominator via ones-blocks, numerator via class-value blocks) in PSUM.
  A DRAM bounce regroups the [16, cc] PSUM tile to [128, *] so the DVE
  divide/add runs with all partitions active.

  Host then assembles x (index scatter only), gathers x along the sorted
  edge list, and lays edge (x, feature) pairs out slot-major per
  constraint-degree tier so the per-constraint segment sum becomes a
  binary tree of contiguous fp16 tensor_tensor adds (2x DVE mode).

  Launch 2: per chunk w = xg * ft (fp16, in place), tree-reduce to Ax,
  then one stats pass: viol = relu(Ax - bias), sum / max / count.
"""

import sys

sys.path.insert(0, "/opt/trn_rl_repo")

import numpy as np

import concourse.bass as bass
import concourse.mybir as mybir
from concourse.bass_utils import run_bass_kernel_spmd

P = 128
NCORES = 8
NBINS = P * NCORES
LAMBDA_MEAN, LAMBDA_MAX = 1.0, 0.1
BIAS_COL = 1
LP_SOL_COL = 8
BIG_BIAS = 60000.0          # fp16-safe "never violated" bias for padding segs
CNT_THR = 1e-6
F16 = mybir.dt.float16
F32 = mybir.dt.float32

# phase-1 geometry
P1_CC = 528                 # columns per chunk (rows = 8 per column)
P1_JB2 = 4                  # hop2 column blocks per chunk (partition packing)
P1_GRP = 3                  # chunks per PSUM group (offsets 0/32/64)
P1_QP = 96                  # partitions used by regrouped D/N tiles
# phase-2 chunking
CH_TARGET = 4352            # target stream elems / partition / chunk
MIN_TIER = 6 * NBINS        # merge degree tiers smaller than this

# most recent build params, for the test harness
LAST_ROWS_PP = None
LAST_P2_ARGS = None


# --------------------------------------------------------------------------
# phase 1: expected = (softmax(logits) @ [0..C)) + offsets
# --------------------------------------------------------------------------
def _build_phase1(params):
    """Chunks of cc columns; groups of 3 chunks fill a [128, cc] PSUM tile
    at partition offsets 0/32/64 (rows 96.. stay zero).  DVE evacuates each
    group to SBUF, a DRAM bounce regroups D/N onto 96 partitions, DVE
    divides and adds offsets."""
    global LAST_ROWS_PP
    LAST_ROWS_PP = params
    nch, cc = params
    grp = P1_GRP
    qp = P1_QP
    assert nch % grp == 0
    ngrp = nch // grp
    jb2 = P1_JB2
    jr = cc // jb2
    half = cc // 2
    ncol = nch * cc

    nc = bass.Bass()
    lg = nc.declare_dram_parameter("logits", [P, ncol], F16, isOutput=False)
    wp = nc.declare_dram_parameter("wmat", [P, 32], F16, isOutput=False)
    op = nc.declare_dram_parameter("offs", [qp, ngrp * jr], F32, isOutput=False)
    ex = nc.declare_dram_parameter("expected", [qp, ngrp * jr], F32, isOutput=True)
    scr = nc.dram_tensor("scratch1", [ngrp, P, cc], F32, kind="Internal")

    with (
        nc.sbuf_tensor([P, 2, grp, cc], F16) as tlg,   # group-buffered logits
        nc.sbuf_tensor([P, 2, grp, cc], F16) as te,    # exp(logits)
        nc.sbuf_tensor([P, 32], F16) as tw,
        nc.sbuf_tensor([P, 2, cc], F32) as sbc,        # psum evacuation
        nc.sbuf_tensor([P, 2, 2, jr], F32) as tdn,     # regrouped [D, N]
        nc.sbuf_tensor([P, jr], F32) as trec,
        nc.sbuf_tensor([P, ngrp * jr], F32) as toffs,
        nc.sbuf_tensor([P, ngrp * jr], F32) as ebuf,
        nc.psum_tensor([P, cc], F32) as ps0,
        nc.psum_tensor([P, cc], F32) as ps1,
        nc.Block() as block,
        nc.semaphore("wsem") as wsem,
        nc.semaphore("lsem") as lsem,
        nc.semaphore("esem") as esem,
        nc.semaphore("msem") as msem,
        nc.semaphore("csem") as csem,
        nc.semaphore("ssem") as ssem,
        nc.semaphore("dsem") as dsem,
        nc.semaphore("vsem") as vsem,
        nc.semaphore("osem") as osem,
    ):
        ps = [ps0, ps1]

        @block.sync
        def _(sync):
            sync.dma_start(out=tw[:], in_=wp[:]).then_inc(wsem, 16)
            sync.dma_start(
                out=toffs[0:qp, :], in_=op[:]
            ).then_inc(wsem, 16)
            for gi in range(ngrp):
                if gi >= 2:
                    sync.wait_ge(esem, gi - 1)
                sync.dma_start(
                    out=tlg[:, gi % 2, :, :],
                    in_=lg[:, grp * gi * cc : grp * (gi + 1) * cc],
                ).then_inc(lsem, 16)
            for gi in range(ngrp):
                sync.wait_ge(csem, gi + 1)
                sync.dma_start(out=scr[gi], in_=sbc[:, gi % 2, :]).then_inc(
                    ssem, 16
                )
            sync.wait_ge(vsem, ngrp)
            sync.dma_start(out=ex[:], in_=ebuf[0:qp, :]).then_inc(osem, 16)
            sync.wait_ge(osem, 16)

        @block.scalar
        def _(scalar):
            for gi in range(ngrp):
                scalar.wait_ge(lsem, 16 * (gi + 1))
                if gi >= 2:
                    scalar.wait_ge(msem, grp * (gi - 1))   # te group reuse
                nc.scalar.activation(
                    out=te[:, gi % 2, :, :].rearrange("p a b -> p (a b)"),
                    in_=tlg[:, gi % 2, :, :].rearrange("p a b -> p (a b)"),
                    func=mybir.ActivationFunctionType.Exp,
                ).then_inc(esem, 1)

        @block.tensor
        def _(tensor):
            tensor.wait_ge(wsem, 16)
            for c in range(nch):
                gi, t = c // grp, c % grp
                tensor.wait_ge(esem, gi + 1)
                if gi >= 2:
                    tensor.wait_ge(csem, gi - 1)   # psum group reuse
                pb = ps[gi % 2]
                eb = te[:, gi % 2, t, :]
                nc.tensor.matmul(
                    out=pb[32 * t : 32 * t + 32, 0:half],
                    lhsT=tw[:], rhs=eb[0:P, 0:half],
                    start=True, stop=True,
                )
                nc.tensor.matmul(
                    out=pb[32 * t : 32 * t + 32, half:cc],
                    lhsT=tw[:], rhs=eb[0:P, half:cc],
                    start=True, stop=True,
                ).then_inc(msem, 1)

        @block.gpsimd
        def _(gpsimd):
            for gi in range(ngrp):
                gpsimd.wait_ge(ssem, 16 * (gi + 1))
                if gi >= 2:
                    gpsimd.wait_ge(vsem, gi - 1)   # tdn reuse
                # partition q=(t*8+g)*4+jb2 <- scr row 32t + 8h + g,
                # col block jb2; one DMA per h so each AP is 3 dims.
                for h in range(2):
                    src = bass.AP(
                        scr,
                        gi * P * cc + h * 8 * cc,
                        [[32 * cc, grp], [jr, 8 * jb2], [1, jr]],
                    )
                    gpsimd.dma_start(
                        out=tdn[0:qp, gi % 2, h, :], in_=src
                    ).then_inc(dsem, 16)

        def _div(vector, gi):
            vector.wait_ge(dsem, 32 * (gi + 1))
            nc.vector.reciprocal(
                out=trec[0:qp, :], in_=tdn[0:qp, gi % 2, 0, :]
            )
            nc.vector.tensor_tensor(
                out=ebuf[0:qp, gi * jr : (gi + 1) * jr],
                in0=tdn[0:qp, gi % 2, 1, :], in1=trec[0:qp, :],
                op=mybir.AluOpType.mult,
            )
            nc.vector.tensor_tensor(
                out=ebuf[0:qp, gi * jr : (gi + 1) * jr],
                in0=ebuf[0:qp, gi * jr : (gi + 1) * jr],
                in1=toffs[0:qp, gi * jr : (gi + 1) * jr],
                op=mybir.AluOpType.add,
            ).then_inc(vsem, 1)

        @block.vector
        def _(vector):
            for gi in range(ngrp):
                vector.wait_ge(msem, grp * (gi + 1))
                if gi >= 2:
                    vector.wait_ge(ssem, 16 * (gi - 1))   # sbc reuse
                nc.vector.tensor_copy(
                    out=sbc[:, gi % 2, :], in_=ps[gi % 2][:, :]
                ).then_inc(csem, 1)
                _div(vector, gi)

    return nc


# --------------------------------------------------------------------------
# phase 2: w = xg*ft, tree segment-sum -> Ax, viol stats
# --------------------------------------------------------------------------
def _build_phase2(tiers):
    """tiers: tuple of (L, Spad, Rc) per degree tier; chunk = Rc ranks."""
    global LAST_P2_ARGS
    LAST_P2_ARGS = (tiers,)
    nc = bass.Bass()
    ax_tot = sum(s for _, s, _ in tiers)
    chunks = []          # (tier_idx, chunk_idx, axbase)
    axb = 0
    for t, (L, Spad, Rc) in enumerate(tiers):
        for ch in range(Spad // Rc):
            chunks.append((t, ch, axb + ch * Rc))
        axb += Spad
    chmax = max(2 * L * Rc for L, _, Rc in tiers)

    xs = [
        nc.declare_dram_parameter(f"st{t}", [P, Spad * 2 * L], F16, False)
        for t, (L, Spad, Rc) in enumerate(tiers)
    ]
    bs = nc.declare_dram_parameter("bias", [P, ax_tot], F16, isOutput=False)
    out_p = nc.declare_dram_parameter("partials", [P, 4], F32, isOutput=True)

    with (
        nc.sbuf_tensor([P, 2, chmax], F16) as tst,
        nc.sbuf_tensor([P, ax_tot], F16) as tax,
        nc.sbuf_tensor([P, ax_tot], F16) as tb,
        nc.sbuf_tensor([P, ax_tot], F16) as tv,
        nc.sbuf_tensor([P, 1], F16) as tm16,
        nc.sbuf_tensor([P, 4], F32) as tout,
        nc.Block() as block,
        nc.semaphore("bsem") as bsem,
        nc.semaphore("pa") as pa,
        nc.semaphore("vs") as vs,
        nc.semaphore("fsem") as fsem,
        nc.semaphore("osem") as osem,
    ):

        @block.sync
        def _(sync):
            sync.dma_start(out=tb[:], in_=bs[:]).then_inc(bsem, 16)
            for i, (t, ch, _axb) in enumerate(chunks):
                L, Spad, Rc = tiers[t]
                sz = 2 * L * Rc
                if i >= 2:
                    sync.wait_ge(vs, i - 1)
                sync.dma_start(
                    out=tst[:, i % 2, 0:sz], in_=xs[t][:, ch * sz : (ch + 1) * sz]
                ).then_inc(pa, 16)
            sync.wait_ge(fsem, 1)
            sync.dma_start(out=out_p[:], in_=tout[:]).then_inc(osem, 16)
            sync.wait_ge(osem, 16)

        @block.vector
        def _(vector):
            for i, (t, ch, axb_c) in enumerate(chunks):
                L, Spad, Rc = tiers[t]
                m = L * Rc
                vector.wait_ge(pa, 16 * (i + 1))
                w = tst[:, i % 2, :]
                nc.vector.tensor_tensor(
                    out=w[0:P, 0:m], in0=w[0:P, 0:m], in1=w[0:P, m : 2 * m],
                    op=mybir.AluOpType.mult,
                )
                h = L
                while h > 1:
                    if h % 2 == 1:
                        nc.vector.tensor_tensor(
                            out=w[0:P, 0:Rc],
                            in0=w[0:P, 0:Rc],
                            in1=w[0:P, (h - 1) * Rc : h * Rc],
                            op=mybir.AluOpType.add,
                        )
                        h -= 1
                    hf = (h // 2) * Rc
                    if h == 2:
                        nc.vector.tensor_tensor(
                            out=tax[:, axb_c : axb_c + Rc],
                            in0=w[0:P, 0:Rc], in1=w[0:P, Rc : 2 * Rc],
                            op=mybir.AluOpType.add,
                        ).then_inc(vs, 1)
                    else:
                        nc.vector.tensor_tensor(
                            out=w[0:P, 0:hf], in0=w[0:P, 0:hf],
                            in1=w[0:P, hf : 2 * hf],
                            op=mybir.AluOpType.add,
                        )
                    h //= 2
            # ---- stats over all segments ----
            vector.wait_ge(bsem, 16)
            nc.vector.tensor_tensor(
                out=tv[:], in0=tax[:], in1=tb[:], op=mybir.AluOpType.subtract
            )
            nc.vector.tensor_scalar_max(out=tv[:], in0=tv[:], scalar1=0.0)
            nc.vector.tensor_reduce(
                out=tout[:, 0:1], in_=tv[:],
                axis=mybir.AxisListType.X, op=mybir.AluOpType.add,
            )
            nc.vector.tensor_reduce(
                out=tm16[:], in_=tv[:],
                axis=mybir.AxisListType.X, op=mybir.AluOpType.max,
            )
            nc.vector.tensor_copy(out=tout[:, 1:2], in_=tm16[:])
            nc.vector.tensor_scalar(
                out=tv[:], in0=tv[:], scalar1=CNT_THR, scalar2=None,
                op0=mybir.AluOpType.is_gt,
            )
            nc.vector.tensor_reduce(
                out=tout[:, 2:3], in_=tv[:],
                axis=mybir.AxisListType.X, op=mybir.AluOpType.add,
            )
            nc.vector.tensor_copy(out=tout[:, 3:4], in_=tout[:, 2:3]).then_inc(
                fsem, 1
            )

    return nc


def _ceil_div(a, b):
    return -(-a // b)


# --------------------------------------------------------------------------
# host-side layout prep (index shuffling only)
# --------------------------------------------------------------------------
def _prep_phase1(logits, offsets):
    """Pack per-core class-major logits grids + offsets; return arrays."""
    ns, ccls = logits.shape
    assert ccls == 16
    rows_core = _ceil_div(ns, NCORES)
    ncol_need = _ceil_div(rows_core, 8)
    cc = P1_CC
    nch = _ceil_div(_ceil_div(ncol_need, cc), P1_GRP) * P1_GRP
    ncol = nch * cc
    rows_cap = 8 * ncol
    ngrp = nch // P1_GRP
    jb2 = P1_JB2
    jr = cc // jb2

    # weight block: cols 0..7 ones per group, 8..15 class values, 16..31 zero
    W = np.zeros((P, 32), dtype=np.float16)
    pidx = np.arange(P)
    g = pidx // 16
    c = pidx % 16
    W[pidx, g] = 1.0
    W[pidx, 8 + g] = c.astype(np.float16)

    lgs, offs_packed = [], []
    for core in range(NCORES):
        lo, hi = core * rows_core, min((core + 1) * rows_core, ns)
        lgp = np.zeros((rows_cap, ccls), dtype=np.float16)
        lgp[: hi - lo] = logits[lo:hi].astype(np.float16)
        # partition p = g*16 + cls, column j; row r = g*ncol + j
        lgs.append(
            np.ascontiguousarray(
                lgp.reshape(8, ncol, ccls).transpose(0, 2, 1).reshape(P, ncol)
            )
        )
        ofp = np.zeros(rows_cap, dtype=np.float32)
        ofp[: hi - lo] = offsets[lo:hi]
        # q = (t*8+g)*4 + jb2, col = gi*jr + jr_i
        # row = g*ncol + (grp*gi+t)*cc + jb2*jr + jr_i
        offs_packed.append(
            np.ascontiguousarray(
                ofp.reshape(8, ngrp, P1_GRP, jb2, jr)
                .transpose(2, 0, 3, 1, 4)
                .reshape(P1_QP, ngrp * jr)
            )
        )
    return (nch, cc), W, lgs, offs_packed, rows_core, rows_cap


def _unpack_expected(e_packed, nch, jr, rows_cap, nrows):
    # e[q=(t*8+g)*4+jb2, gi*jr+jr_i] -> row g*ncol + (grp*gi+t)*cc + jb2*jr+jr_i
    ngrp = nch // P1_GRP
    return (
        e_packed.reshape(P1_GRP, 8, P1_JB2, ngrp, jr)
        .transpose(1, 3, 0, 2, 4)
        .reshape(rows_cap)[:nrows]
    )


def _prep_phase2(con, var, feat, bias, n_con):
    """Sort edges, tier by degree, slot-major layout. Returns metadata +
    per-tier (core-major) index/feature arrays; xg filled later."""
    ne = con.shape[0]
    deg = np.bincount(con, minlength=n_con)
    order = np.argsort(con, kind="stable")
    run_start = np.zeros(n_con + 1, dtype=np.int64)
    np.cumsum(deg, out=run_start[1:])
    con_sorted = con[order]
    off_in_run = np.arange(ne, dtype=np.int64) - run_start[con_sorted]
    var_sorted = var[order]
    feat_sorted = feat[order]

    maxdeg = int(deg.max()) if ne else 1
    cand = list(range(16, 68, 4))
    if maxdeg > cand[-1]:
        cand.append(_ceil_div(maxdeg, 4) * 4)
    cand = np.asarray(cand, dtype=np.int64)
    t_cand = np.searchsorted(cand, deg, side="left")
    cnt = np.bincount(t_cand, minlength=len(cand))
    # merge small tiers upward into the next stride
    keep = []
    acc = 0
    remap = np.zeros(len(cand), dtype=np.int64)
    for si in range(len(cand)):
        acc += cnt[si]
        remap[si] = len(keep)
        if (acc >= MIN_TIER) or (si == len(cand) - 1 and acc > 0):
            keep.append(int(cand[si]))
            acc = 0
    t_of_seg = remap[t_cand]

    tiers = []
    tier_data = []
    axb = 0
    dense = np.zeros(len(keep), dtype=np.int64)
    for t, L in enumerate(keep):
        segs = np.nonzero(t_of_seg == t)[0]
        n_t = segs.shape[0]
        if n_t == 0:
            dense[t] = -1
            continue
        dense[t] = len(tiers)
        S_t = _ceil_div(n_t, NBINS)
        nch_t = max(1, int(round(S_t * 2 * L / CH_TARGET)))
        nch_t = min(nch_t, S_t)
        Rc = _ceil_div(S_t, nch_t)
        Spad = nch_t * Rc
        k_of_con = np.full(n_con, -1, dtype=np.int64)
        k_of_con[segs] = np.arange(n_t)
        tiers.append((L, Spad, Rc))
        tier_data.append((segs, k_of_con, axb))
        axb += Spad
    t_of_seg = dense[t_of_seg]

    ax_tot = axb
    bias_arr = np.full((NCORES, P, ax_tot), BIG_BIAS, dtype=np.float16)
    for (L, Spad, Rc), (segs, k_of_con, axb) in zip(tiers, tier_data):
        k = k_of_con[segs]
        bb = k % NBINS
        r = k // NBINS
        bias_arr[bb // P, bb % P, axb + r] = bias[segs].astype(np.float16)

    return (
        tiers,
        tier_data,
        ax_tot,
        bias_arr,
        con_sorted,
        off_in_run,
        var_sorted,
        feat_sorted,
        t_of_seg,
    )


def _fill_streams(tiers, tier_data, t_of_seg, con_sorted, off_in_run,
                  var_sorted, feat_sorted, x16):
    """Build per-tier interleaved (xg, ft) fp16 streams, slot-major."""
    e_tier = t_of_seg[con_sorted]
    streams = []
    for t, ((L, Spad, Rc), (segs, k_of_con, axb)) in enumerate(
        zip(tiers, tier_data)
    ):
        sel = np.nonzero(e_tier == t)[0]
        cs = con_sorted[sel]
        slot = off_in_run[sel]
        k = k_of_con[cs]
        b = k % NBINS
        r = k // NBINS
        core = b // P
        part = b % P
        ch = r // Rc
        rin = r % Rc
        base = ch * (2 * L * Rc)
        col_x = base + slot * Rc + rin
        col_f = base + (L + slot) * Rc + rin
        width = Spad * 2 * L
        arr = np.zeros(NCORES * P * width, dtype=np.float16)
        flat_base = (core * P + part) * width
        arr[flat_base + col_x] = x16[var_sorted[sel]]
        arr[flat_base + col_f] = feat_sorted[sel].astype(np.float16)
        streams.append(arr.reshape(NCORES, P, width))
    return streams


# --------------------------------------------------------------------------
def kernel(**inputs) -> tuple:
    prob_bin = np.asarray(inputs["prob_bin"], dtype=np.float32)
    logits = np.asarray(inputs["logits_int_small"], dtype=np.float32)
    offsets = np.asarray(inputs["int_small_offsets"], dtype=np.float32)
    pred_l = np.asarray(inputs["pred_int_large"], dtype=np.float32)
    feat = np.asarray(inputs["edge_features"], dtype=np.float32).reshape(-1)
    cfeat = np.asarray(inputs["constraint_features"], dtype=np.float32)
    vfeat = np.asarray(inputs["variable_features"], dtype=np.float32)
    idx_bin = np.asarray(inputs["idx_bin"], dtype=np.int64)
    idx_s = np.asarray(inputs["idx_int_small"], dtype=np.int64)
    idx_l = np.asarray(inputs["idx_int_large"], dtype=np.int64)
    var_types = np.asarray(inputs["var_types"], dtype=np.int64)
    ei = np.asarray(inputs["edge_indices"], dtype=np.int64)
    n_vars = int(inputs["n_vars"])

    n_con = cfeat.shape[0]
    ns = logits.shape[0]
    bias = np.ascontiguousarray(cfeat[:, BIAS_COL])
    lp_vals = np.ascontiguousarray(vfeat[:, LP_SOL_COL])

    # ---------------- launch 1 ----------------
    (nch, cc), W, lgs, offs_packed, rows_core, rows_cap = _prep_phase1(
        logits, offsets
    )
    jr = cc // P1_JB2
    nc1 = _build_phase1((nch, cc))
    in1 = [
        {"logits": lgs[c], "wmat": W, "offs": offs_packed[c]}
        for c in range(NCORES)
    ]
    res1 = run_bass_kernel_spmd(nc1, in1, list(range(NCORES)))
    expected = np.concatenate(
        [
            _unpack_expected(
                res1.results[c]["expected"], nch, jr, rows_cap, rows_core
            )
            for c in range(NCORES)
        ]
    )[:ns]

    # ---------------- host: assemble x ----------------
    xfull = np.zeros(n_vars, dtype=np.float32)
    xfull[idx_bin] = prob_bin[:, 0]
    xfull[idx_s] = expected
    xfull[idx_l] = pred_l[:, 0]
    xfull = np.where(var_types == 0, lp_vals, xfull)
    x16 = xfull.astype(np.float16)

    # ---------------- launch 2 ----------------
    (
        tiers, tier_data, ax_tot, bias_arr, con_sorted, off_in_run,
        var_sorted, feat_sorted, t_of_seg,
    ) = _prep_phase2(ei[0], ei[1], feat, bias, n_con)
    streams = _fill_streams(
        tiers, tier_data, t_of_seg, con_sorted, off_in_run, var_sorted,
        feat_sorted, x16,
    )
    nc2 = _build_phase2(tuple(tiers))
    in2 = []
    for c in range(NCORES):
        m = {f"st{t}": streams[t][c] for t in range(len(tiers))}
        m["bias"] = bias_arr[c]
        in2.append(m)
    res2 = run_bass_kernel_spmd(nc2, in2, list(range(NCORES)))

    parts = np.stack([res2.results[c]["partials"] for c in range(NCORES)])
    vsum = parts[:, :, 0].astype(np.float64).sum()
    vmax = np.float32(parts[:, :, 1].max())
    vcnt = np.int64(round(float(parts[:, :, 2].astype(np.float64).sum())))
    mean_viol = np.float32(vsum / np.float64(n_con))
    penalty = np.float32(
        np.float32(LAMBDA_MEAN) * mean_viol + np.float32(LAMBDA_MAX) * vmax
    )
    return penalty, mean_viol, vmax, vcnt


# revision 24
# speedup vs baseline: 1.8074x; 1.4512x over previous
"""Trainium2 Bass kernel for ConstraintViolationLoss (GNN message passing).

Two launches on 8 NeuronCores (SPMD), fp16 data streams:

  Launch 1 (softmax expected-value head): logits are laid out class-on-
  partition ([128, ncol] tiles, partition p = 16*g + c holding class c of
  row-group g), ACT computes exp in fp16, and ONE PE matmul against a
  constant [128, 16] weight block produces both softmax sums per row
  (denominator via ones-blocks, numerator via class-value blocks) in PSUM.
  A DRAM bounce regroups the [16, cc] PSUM tile to [128, *] so the DVE
  divide/add runs with all partitions active.

  Host then assembles x (index scatter only), gathers x along the sorted
  edge list, and lays edge (x, feature) pairs out slot-major per
  constraint-degree tier so the per-constraint segment sum becomes a
  binary tree of contiguous fp16 tensor_tensor adds (2x DVE mode).

  Launch 2: per chunk w = xg * ft (fp16, in place), tree-reduce to Ax,
  then one stats pass: viol = relu(Ax - bias), sum / max / count.
"""

import sys

sys.path.insert(0, "/opt/trn_rl_repo")

import numpy as np

import concourse.bass as bass
import concourse.mybir as mybir
from concourse.bass_utils import run_bass_kernel_spmd

P = 128
NCORES = 8
NBINS = P * NCORES
LAMBDA_MEAN, LAMBDA_MAX = 1.0, 0.1
BIAS_COL = 1
LP_SOL_COL = 8
BIG_BIAS = 60000.0          # fp16-safe "never violated" bias for padding segs
CNT_THR = 1e-6
F16 = mybir.dt.float16
F32 = mybir.dt.float32

# phase-1 geometry
P1_CC = 480                 # columns per chunk (rows = 8 per column)
P1_GRP = 3                  # chunks per PSUM group (row offsets 0/32/64)
P1_TW = 120                 # transpose tile width (out partitions)
# phase-2 chunking
CH_TARGET = 3072            # target stream elems / partition / chunk
NBUF2 = 4                   # stream buffers in phase 2
MIN_TIER = 6 * NBINS        # merge degree tiers smaller than this

# most recent build params, for the test harness
LAST_ROWS_PP = None
LAST_P2_ARGS = None


# --------------------------------------------------------------------------
# phase 1: expected = (softmax(logits) @ [0..C)) + offsets
# --------------------------------------------------------------------------
def _p1_groups(nch):
    gs = []
    left = nch
    while left > 0:
        gs.append(min(P1_GRP, left))
        left -= P1_GRP
    return gs


def _build_phase1(params):
    """Chunk c of group gi writes its [32, cc] matmul output (rows: D_g at
    32t+g, N_g at 32t+8+g) into a shared [128, cc] PSUM tile.  DVE
    evacuates each group to SBUF as fp16, PE transposes 120-column tiles so
    D and N land on the same partition (different free offsets), and DVE
    divides batches of two groups at once."""
    global LAST_ROWS_PP
    LAST_ROWS_PP = params
    nch, cc = params
    gs = _p1_groups(nch)
    ngrp = len(gs)
    cum = np.cumsum(gs).tolist()
    nbatch = _ceil_div(ngrp, 2)
    tw = P1_TW
    ntile = cc // tw            # transpose tiles per group
    dcols = ntile * 2 * 24      # e-columns per group-pair batch
    ncol = nch * cc

    nc = bass.Bass()
    lg = nc.declare_dram_parameter("logits", [P, ncol], F16, isOutput=False)
    wp = nc.declare_dram_parameter("wmat", [P, 32], F16, isOutput=False)
    idp = nc.declare_dram_parameter("ident", [P, 96], F16, isOutput=False)
    op = nc.declare_dram_parameter("offs", [tw, nbatch * dcols], F32, False)
    ex = nc.declare_dram_parameter(
        "expected", [tw, nbatch * dcols], F32, isOutput=True
    )

    with (
        nc.sbuf_tensor([P, 2, P1_GRP * cc], F16) as tlg,
        nc.sbuf_tensor([P, 2, P1_GRP * cc], F16) as te,
        nc.sbuf_tensor([P, 32], F16) as tw_,
        nc.sbuf_tensor([P, 96], F16) as tid,
        nc.sbuf_tensor([P, 2, cc], F16) as sbc,        # psum evacuation
        nc.sbuf_tensor([P, nbatch * dcols], F32) as trec,
        nc.sbuf_tensor([P, nbatch * dcols], F32) as toffs,
        nc.sbuf_tensor([P, nbatch * dcols], F32) as ebuf,
        nc.psum_tensor([P, cc], F32) as ps0,
        nc.psum_tensor([P, cc], F32) as ps1,
        nc.psum_tensor([P, 2 * ntile, 96], F16) as pt0,
        nc.psum_tensor([P, 2 * ntile, 96], F16) as pt1,
        nc.Block() as block,
        nc.semaphore("wsem") as wsem,
        nc.semaphore("lsem") as lsem,
        nc.semaphore("esem") as esem,
        nc.semaphore("msem") as msem,
        nc.semaphore("csem") as csem,
        nc.semaphore("tsem") as tsem,
        nc.semaphore("vsem") as vsem,
        nc.semaphore("osem") as osem,
    ):
        ps = [ps0, ps1]
        pt = [pt0, pt1]

        @block.sync
        def _(sync):
            sync.dma_start(out=tw_[:], in_=wp[:]).then_inc(wsem, 16)
            sync.dma_start(out=tid[:], in_=idp[:]).then_inc(wsem, 16)
            sync.dma_start(out=toffs[0:tw, :], in_=op[:]).then_inc(wsem, 16)
            for gi in range(ngrp):
                if gi >= 2:
                    sync.wait_ge(esem, gi - 1)    # tlg buffer reuse
                c0 = cum[gi] - gs[gi]
                sync.dma_start(
                    out=tlg[:, gi % 2, 0 : gs[gi] * cc],
                    in_=lg[:, c0 * cc : cum[gi] * cc],
                ).then_inc(lsem, 16)
            sync.wait_ge(vsem, nbatch)
            sync.dma_start(out=ex[:], in_=ebuf[0:tw, :]).then_inc(osem, 16)
            sync.wait_ge(osem, 16)

        @block.scalar
        def _(scalar):
            for gi in range(ngrp):
                scalar.wait_ge(lsem, 16 * (gi + 1))
                if gi >= 2:
                    scalar.wait_ge(msem, cum[gi - 2])   # te buffer reuse
                nc.scalar.activation(
                    out=te[:, gi % 2, 0 : gs[gi] * cc],
                    in_=tlg[:, gi % 2, 0 : gs[gi] * cc],
                    func=mybir.ActivationFunctionType.Exp,
                ).then_inc(esem, 1)

        def _mm_group(tensor, gi):
            tensor.wait_ge(esem, gi + 1)
            if gi >= 2:
                tensor.wait_ge(csem, gi - 1)   # psum tile reuse
            for t in range(gs[gi]):
                nc.tensor.matmul(
                    out=ps[gi % 2][32 * t : 32 * t + 32, :],
                    lhsT=tw_[:],
                    rhs=te[:, gi % 2, t * cc : (t + 1) * cc],
                    start=True, stop=True,
                ).then_inc(msem, 1)

        def _tr_group(tensor, gi):
            tensor.wait_ge(csem, gi + 1)
            for k in range(ntile):
                nc.tensor.transpose(
                    out=pt[gi // 2][0:tw, (gi % 2) * ntile + k, :],
                    in_=sbc[0:96, gi % 2, k * tw : (k + 1) * tw],
                    identity=tid[0:96, :],
                ).then_inc(tsem, 1)

        @block.tensor
        def _(tensor):
            tensor.wait_ge(wsem, 32)
            _mm_group(tensor, 0)
            for gi in range(1, ngrp):
                _mm_group(tensor, gi)
                _tr_group(tensor, gi - 1)
            _tr_group(tensor, ngrp - 1)

        def _div(vector, bi, glast):
            vector.wait_ge(tsem, ntile * (glast + 1))
            pb = pt[bi][0:tw, :, :].rearrange("p a (t m) -> p a t m", m=32)
            sl = slice(bi * dcols, (bi + 1) * dcols)
            nc.vector.reciprocal(
                out=trec[0:tw, sl], in_=pb[:, :, :, 0:8]
            )
            nc.vector.tensor_tensor(
                out=ebuf[0:tw, sl], in0=pb[:, :, :, 8:16],
                in1=trec[0:tw, sl], op=mybir.AluOpType.mult,
            )
            nc.vector.tensor_tensor(
                out=ebuf[0:tw, sl], in0=ebuf[0:tw, sl],
                in1=toffs[0:tw, sl], op=mybir.AluOpType.add,
            ).then_inc(vsem, 1)

        @block.vector
        def _(vector):
            vector.wait_ge(wsem, 48)
            for gi in range(ngrp):
                vector.wait_ge(msem, cum[gi])
                if gi >= 2:
                    vector.wait_ge(tsem, ntile * (gi - 1))   # sbc reuse
                nc.vector.tensor_copy(
                    out=sbc[0:96, gi % 2, :], in_=ps[gi % 2][0:96, :]
                ).then_inc(csem, 1)
                if gi % 2 == 1:
                    _div(vector, gi // 2, gi)
            if ngrp % 2 == 1:
                _div(vector, ngrp // 2, ngrp - 1)

    return nc


# --------------------------------------------------------------------------
# phase 2: w = xg*ft, tree segment-sum -> Ax, viol stats
# --------------------------------------------------------------------------
def _build_phase2(tiers):
    """tiers: tuple of (L, Spad, Rc) per degree tier; chunk = Rc ranks."""
    global LAST_P2_ARGS
    LAST_P2_ARGS = (tiers,)
    nc = bass.Bass()
    ax_tot = sum(s for _, s, _ in tiers)
    chunks = []          # (tier_idx, chunk_idx, axbase)
    axb = 0
    for t, (L, Spad, Rc) in enumerate(tiers):
        for ch in range(Spad // Rc):
            chunks.append((t, ch, axb + ch * Rc))
        axb += Spad
    chmax = max(2 * L * Rc for L, _, Rc in tiers)

    xs = [
        nc.declare_dram_parameter(f"st{t}", [P, Spad * 2 * L], F16, False)
        for t, (L, Spad, Rc) in enumerate(tiers)
    ]
    bs = nc.declare_dram_parameter("bias", [P, ax_tot], F16, isOutput=False)
    out_p = nc.declare_dram_parameter("partials", [P, 4], F32, isOutput=True)

    with (
        nc.sbuf_tensor([P, NBUF2, chmax], F16) as tst,
        nc.sbuf_tensor([P, ax_tot], F16) as tax,
        nc.sbuf_tensor([P, ax_tot], F16) as tb,
        nc.sbuf_tensor([P, ax_tot], F16) as tv,
        nc.sbuf_tensor([P, 1], F16) as tm16,
        nc.sbuf_tensor([P, 4], F32) as tout,
        nc.Block() as block,
        nc.semaphore("bsem") as bsem,
        nc.semaphore("pa") as pa,
        nc.semaphore("vs") as vs,
        nc.semaphore("fsem") as fsem,
        nc.semaphore("osem") as osem,
    ):

        @block.sync
        def _(sync):
            sync.dma_start(out=tb[:], in_=bs[:]).then_inc(bsem, 16)
            for i, (t, ch, _axb) in enumerate(chunks):
                L, Spad, Rc = tiers[t]
                sz = 2 * L * Rc
                if i >= NBUF2:
                    sync.wait_ge(vs, i - NBUF2 + 1)
                sync.dma_start(
                    out=tst[:, i % NBUF2, 0:sz],
                    in_=xs[t][:, ch * sz : (ch + 1) * sz],
                ).then_inc(pa, 16)
            sync.wait_ge(fsem, 1)
            sync.dma_start(out=out_p[:], in_=tout[:]).then_inc(osem, 16)
            sync.wait_ge(osem, 16)

        @block.vector
        def _(vector):
            for i, (t, ch, axb_c) in enumerate(chunks):
                L, Spad, Rc = tiers[t]
                m = L * Rc
                vector.wait_ge(pa, 16 * (i + 1))
                w = tst[:, i % NBUF2, :]
                nc.vector.tensor_tensor(
                    out=w[0:P, 0:m], in0=w[0:P, 0:m], in1=w[0:P, m : 2 * m],
                    op=mybir.AluOpType.mult,
                )
                h = L
                while h > 1:
                    if h % 2 == 1:
                        nc.vector.tensor_tensor(
                            out=w[0:P, 0:Rc],
                            in0=w[0:P, 0:Rc],
                            in1=w[0:P, (h - 1) * Rc : h * Rc],
                            op=mybir.AluOpType.add,
                        )
                        h -= 1
                    hf = (h // 2) * Rc
                    if h == 2:
                        nc.vector.tensor_tensor(
                            out=tax[:, axb_c : axb_c + Rc],
                            in0=w[0:P, 0:Rc], in1=w[0:P, Rc : 2 * Rc],
                            op=mybir.AluOpType.add,
                        ).then_inc(vs, 1)
                    else:
                        nc.vector.tensor_tensor(
                            out=w[0:P, 0:hf], in0=w[0:P, 0:hf],
                            in1=w[0:P, hf : 2 * hf],
                            op=mybir.AluOpType.add,
                        )
                    h //= 2
            # ---- stats over all segments ----
            vector.wait_ge(bsem, 16)
            nc.vector.tensor_tensor(
                out=tv[:], in0=tax[:], in1=tb[:], op=mybir.AluOpType.subtract
            )
            nc.vector.tensor_scalar_max(out=tv[:], in0=tv[:], scalar1=0.0)
            nc.vector.tensor_reduce(
                out=tout[:, 0:1], in_=tv[:],
                axis=mybir.AxisListType.X, op=mybir.AluOpType.add,
            )
            nc.vector.tensor_reduce(
                out=tm16[:], in_=tv[:],
                axis=mybir.AxisListType.X, op=mybir.AluOpType.max,
            )
            nc.vector.tensor_copy(out=tout[:, 1:2], in_=tm16[:])
            nc.vector.tensor_scalar(
                out=tv[:], in0=tv[:], scalar1=CNT_THR, scalar2=None,
                op0=mybir.AluOpType.is_gt,
            )
            nc.vector.tensor_reduce(
                out=tout[:, 2:3], in_=tv[:],
                axis=mybir.AxisListType.X, op=mybir.AluOpType.add,
            )
            nc.vector.tensor_copy(out=tout[:, 3:4], in_=tout[:, 2:3]).then_inc(
                fsem, 1
            )

    return nc


def _ceil_div(a, b):
    return -(-a // b)


# --------------------------------------------------------------------------
# host-side layout prep (index shuffling only)
# --------------------------------------------------------------------------
def _p1_rowmap(nch, cc):
    """row_of[j, bi, ti, t, g] -> packed row index (or -1 if the slot is
    junk), matching the device's e/offs column order col = bi*dcols +
    ti*24 + t*8 + g with partition j."""
    gs = _p1_groups(nch)
    ngrp = len(gs)
    nbatch = _ceil_div(ngrp, 2)
    tw = P1_TW
    ntile = cc // tw
    ncol = nch * cc
    j, bi, ti, t, g = np.meshgrid(
        np.arange(tw), np.arange(nbatch), np.arange(2 * ntile),
        np.arange(P1_GRP), np.arange(8), indexing="ij",
    )
    gi = bi * 2 + ti // ntile
    k4 = ti % ntile
    valid = (gi < ngrp) & (t < np.asarray(gs + [0])[np.minimum(gi, ngrp)])
    chunk = np.cumsum([0] + gs)[np.minimum(gi, ngrp - 1)] + t
    row = g * ncol + chunk * cc + k4 * tw + j
    row = np.where(valid, row, -1)
    return row, ncol


def _prep_phase1(logits, offsets):
    """Pack per-core class-major logits grids + offsets; return arrays."""
    ns, ccls = logits.shape
    assert ccls == 16
    rows_core = _ceil_div(ns, NCORES)
    ncol_need = _ceil_div(rows_core, 8)
    cc = P1_CC
    nch = _ceil_div(ncol_need, cc)
    ncol = nch * cc
    rows_cap = 8 * ncol
    tw = P1_TW

    # weight block: cols 0..7 ones per group, 8..15 class values, 16..31 zero
    W = np.zeros((P, 32), dtype=np.float16)
    pidx = np.arange(P)
    g = pidx // 16
    c = pidx % 16
    W[pidx, g] = 1.0
    W[pidx, 8 + g] = c.astype(np.float16)
    ident = np.zeros((P, 96), dtype=np.float16)
    ident[np.arange(96), np.arange(96)] = 1.0

    row_of, _ = _p1_rowmap(nch, cc)
    flat = row_of.reshape(tw, -1)

    lgs, offs_packed = [], []
    for core in range(NCORES):
        lo, hi = core * rows_core, min((core + 1) * rows_core, ns)
        lgp = np.zeros((rows_cap, ccls), dtype=np.float16)
        lgp[: hi - lo] = logits[lo:hi].astype(np.float16)
        # partition p = g*16 + cls, column j; row r = g*ncol + j
        lgs.append(
            np.ascontiguousarray(
                lgp.reshape(8, ncol, ccls).transpose(0, 2, 1).reshape(P, ncol)
            )
        )
        ofp = np.zeros(rows_cap + 1, dtype=np.float32)
        ofp[: hi - lo] = offsets[lo:hi]
        offs_packed.append(np.ascontiguousarray(ofp[flat]))
    return (nch, cc), W, ident, lgs, offs_packed, rows_core, rows_cap


def _unpack_expected(e_packed, nch, cc, rows_cap, nrows):
    row_of, _ = _p1_rowmap(nch, cc)
    flat = row_of.reshape(-1)
    ok = flat >= 0
    out = np.zeros(rows_cap, dtype=np.float32)
    out[flat[ok]] = e_packed.reshape(-1)[ok]
    return out[:nrows]


def _prep_phase2(con, var, feat, bias, n_con):
    """Sort edges, tier by degree, slot-major layout. Returns metadata +
    per-tier (core-major) index/feature arrays; xg filled later."""
    ne = con.shape[0]
    deg = np.bincount(con, minlength=n_con)
    order = np.argsort(con, kind="stable")
    run_start = np.zeros(n_con + 1, dtype=np.int64)
    np.cumsum(deg, out=run_start[1:])
    con_sorted = con[order]
    off_in_run = np.arange(ne, dtype=np.int64) - run_start[con_sorted]
    var_sorted = var[order]
    feat_sorted = feat[order]

    maxdeg = int(deg.max()) if ne else 1
    cand = list(range(16, 68, 4))
    if maxdeg > cand[-1]:
        cand.append(_ceil_div(maxdeg, 4) * 4)
    cand = np.asarray(cand, dtype=np.int64)
    t_cand = np.searchsorted(cand, deg, side="left")
    cnt = np.bincount(t_cand, minlength=len(cand))
    # merge small tiers upward into the next stride
    keep = []
    acc = 0
    remap = np.zeros(len(cand), dtype=np.int64)
    for si in range(len(cand)):
        acc += cnt[si]
        remap[si] = len(keep)
        if (acc >= MIN_TIER) or (si == len(cand) - 1 and acc > 0):
            keep.append(int(cand[si]))
            acc = 0
    t_of_seg = remap[t_cand]

    tiers = []
    tier_data = []
    axb = 0
    dense = np.zeros(len(keep), dtype=np.int64)
    for t, L in enumerate(keep):
        segs = np.nonzero(t_of_seg == t)[0]
        n_t = segs.shape[0]
        if n_t == 0:
            dense[t] = -1
            continue
        dense[t] = len(tiers)
        S_t = _ceil_div(n_t, NBINS)
        nch_t = max(1, int(round(S_t * 2 * L / CH_TARGET)))
        nch_t = min(nch_t, S_t)
        Rc = _ceil_div(S_t, nch_t)
        Spad = nch_t * Rc
        k_of_con = np.full(n_con, -1, dtype=np.int64)
        k_of_con[segs] = np.arange(n_t)
        tiers.append((L, Spad, Rc))
        tier_data.append((segs, k_of_con, axb))
        axb += Spad
    t_of_seg = dense[t_of_seg]

    ax_tot = axb
    bias_arr = np.full((NCORES, P, ax_tot), BIG_BIAS, dtype=np.float16)
    for (L, Spad, Rc), (segs, k_of_con, axb) in zip(tiers, tier_data):
        k = k_of_con[segs]
        bb = k % NBINS
        r = k // NBINS
        bias_arr[bb // P, bb % P, axb + r] = bias[segs].astype(np.float16)

    return (
        tiers,
        tier_data,
        ax_tot,
        bias_arr,
        con_sorted,
        off_in_run,
        var_sorted,
        feat_sorted,
        t_of_seg,
    )


def _fill_streams(tiers, tier_data, t_of_seg, con_sorted, off_in_run,
                  var_sorted, feat_sorted, x16):
    """Build per-tier interleaved (xg, ft) fp16 streams, slot-major."""
    e_tier = t_of_seg[con_sorted]
    streams = []
    for t, ((L, Spad, Rc), (segs, k_of_con, axb)) in enumerate(
        zip(tiers, tier_data)
    ):
        sel = np.nonzero(e_tier == t)[0]
        cs = con_sorted[sel]
        slot = off_in_run[sel]
        k = k_of_con[cs]
        b = k % NBINS
        r = k // NBINS
        core = b // P
        part = b % P
        ch = r // Rc
        rin = r % Rc
        base = ch * (2 * L * Rc)
        col_x = base + slot * Rc + rin
        col_f = base + (L + slot) * Rc + rin
        width = Spad * 2 * L
        arr = np.zeros(NCORES * P * width, dtype=np.float16)
        flat_base = (core * P + part) * width
        arr[flat_base + col_x] = x16[var_sorted[sel]]
        arr[flat_base + col_f] = feat_sorted[sel].astype(np.float16)
        streams.append(arr.reshape(NCORES, P, width))
    return streams


# --------------------------------------------------------------------------
def kernel(**inputs) -> tuple:
    prob_bin = np.asarray(inputs["prob_bin"], dtype=np.float32)
    logits = np.asarray(inputs["logits_int_small"], dtype=np.float32)
    offsets = np.asarray(inputs["int_small_offsets"], dtype=np.float32)
    pred_l = np.asarray(inputs["pred_int_large"], dtype=np.float32)
    feat = np.asarray(inputs["edge_features"], dtype=np.float32).reshape(-1)
    cfeat = np.asarray(inputs["constraint_features"], dtype=np.float32)
    vfeat = np.asarray(inputs["variable_features"], dtype=np.float32)
    idx_bin = np.asarray(inputs["idx_bin"], dtype=np.int64)
    idx_s = np.asarray(inputs["idx_int_small"], dtype=np.int64)
    idx_l = np.asarray(inputs["idx_int_large"], dtype=np.int64)
    var_types = np.asarray(inputs["var_types"], dtype=np.int64)
    ei = np.asarray(inputs["edge_indices"], dtype=np.int64)
    n_vars = int(inputs["n_vars"])

    n_con = cfeat.shape[0]
    ns = logits.shape[0]
    bias = np.ascontiguousarray(cfeat[:, BIAS_COL])
    lp_vals = np.ascontiguousarray(vfeat[:, LP_SOL_COL])

    # ---------------- launch 1 ----------------
    (nch, cc), W, ident, lgs, offs_packed, rows_core, rows_cap = _prep_phase1(
        logits, offsets
    )
    nc1 = _build_phase1((nch, cc))
    in1 = [
        {"logits": lgs[c], "wmat": W, "ident": ident, "offs": offs_packed[c]}
        for c in range(NCORES)
    ]
    res1 = run_bass_kernel_spmd(nc1, in1, list(range(NCORES)))
    expected = np.concatenate(
        [
            _unpack_expected(
                res1.results[c]["expected"], nch, cc, rows_cap, rows_core
            )
            for c in range(NCORES)
        ]
    )[:ns]

    # ---------------- host: assemble x ----------------
    xfull = np.zeros(n_vars, dtype=np.float32)
    xfull[idx_bin] = prob_bin[:, 0]
    xfull[idx_s] = expected
    xfull[idx_l] = pred_l[:, 0]
    xfull = np.where(var_types == 0, lp_vals, xfull)
    x16 = xfull.astype(np.float16)

    # ---------------- launch 2 ----------------
    (
        tiers, tier_data, ax_tot, bias_arr, con_sorted, off_in_run,
        var_sorted, feat_sorted, t_of_seg,
    ) = _prep_phase2(ei[0], ei[1], feat, bias, n_con)
    streams = _fill_streams(
        tiers, tier_data, t_of_seg, con_sorted, off_in_run, var_sorted,
        feat_sorted, x16,
    )
    nc2 = _build_phase2(tuple(tiers))
    in2 = []
    for c in range(NCORES):
        m = {f"st{t}": streams[t][c] for t in range(len(tiers))}
        m["bias"] = bias_arr[c]
        in2.append(m)
    res2 = run_bass_kernel_spmd(nc2, in2, list(range(NCORES)))

    parts = np.stack([res2.results[c]["partials"] for c in range(NCORES)])
    vsum = parts[:, :, 0].astype(np.float64).sum()
    vmax = np.float32(parts[:, :, 1].max())
    vcnt = np.int64(round(float(parts[:, :, 2].astype(np.float64).sum())))
    mean_viol = np.float32(vsum / np.float64(n_con))
    penalty = np.float32(
        np.float32(LAMBDA_MEAN) * mean_viol + np.float32(LAMBDA_MAX) * vmax
    )
    return penalty, mean_viol, vmax, vcnt


# revision 32
# speedup vs baseline: 1.8331x; 1.0142x over previous
"""Trainium2 Bass kernel for ConstraintViolationLoss (GNN message passing).

Two launches on 8 NeuronCores (SPMD), fp16 data streams:

  Launch 1 (softmax expected-value head): logits are laid out class-on-
  partition ([128, ncol] tiles, partition p = 16*g + c holding class c of
  row-group g), ACT computes exp in fp16, and ONE PE matmul against a
  constant [128, 16] weight block produces both softmax sums per row
  (denominator via ones-blocks, numerator via class-value blocks) in PSUM.
  A DRAM bounce regroups the [16, cc] PSUM tile to [128, *] so the DVE
  divide/add runs with all partitions active.

  Host then assembles x (index scatter only), gathers x along the sorted
  edge list, and lays edge (x, feature) pairs out slot-major per
  constraint-degree tier so the per-constraint segment sum becomes a
  binary tree of contiguous fp16 tensor_tensor adds (2x DVE mode).

  Launch 2: per chunk w = xg * ft (fp16, in place), tree-reduce to Ax,
  then one stats pass: viol = relu(Ax - bias), sum / max / count.
"""

import sys

sys.path.insert(0, "/opt/trn_rl_repo")

import numpy as np

import concourse.bass as bass
import concourse.mybir as mybir
from concourse.bass_utils import run_bass_kernel_spmd

P = 128
NCORES = 8
NBINS = P * NCORES
LAMBDA_MEAN, LAMBDA_MAX = 1.0, 0.1
BIAS_COL = 1
LP_SOL_COL = 8
BIG_BIAS = 60000.0          # fp16-safe "never violated" bias for padding segs
CNT_THR = 1e-6
F16 = mybir.dt.float16
F32 = mybir.dt.float32

# phase-1 geometry
P1_CC = 480                 # columns per chunk (rows = 8 per column)
P1_GRP = 3                  # chunks per PSUM group (row offsets 0/32/64)
P1_TW = 120                 # transpose tile width (out partitions)
# phase-2 chunking
CH_TARGET = 3072            # target stream elems / partition / chunk
NBUF2 = 4                   # stream buffers in phase 2
MIN_TIER = 6 * NBINS        # merge degree tiers smaller than this

# most recent build params, for the test harness
LAST_ROWS_PP = None
LAST_P2_ARGS = None


# --------------------------------------------------------------------------
# phase 1: expected = (softmax(logits) @ [0..C)) + offsets
# --------------------------------------------------------------------------
def _p1_groups(nch):
    gs = []
    left = nch
    while left > 0:
        gs.append(min(P1_GRP, left))
        left -= P1_GRP
    return gs


def _build_phase1(params):
    """Chunk c of group gi writes its [32, cc] matmul output (rows: D_g at
    32t+g, N_g at 32t+8+g) into a shared [128, cc] PSUM tile.  DVE
    evacuates each group to SBUF as fp16, PE transposes 120-column tiles so
    D and N land on the same partition (different free offsets), and DVE
    divides batches of two groups at once."""
    global LAST_ROWS_PP
    LAST_ROWS_PP = params
    nch, cc = params
    gs = _p1_groups(nch)
    ngrp = len(gs)
    cum = np.cumsum(gs).tolist()
    nbatch = _ceil_div(ngrp, 2)
    tw = P1_TW
    ntile = cc // tw            # transpose tiles per group
    dcols = ntile * 2 * 24      # e-columns per group-pair batch
    ncol = nch * cc

    nc = bass.Bass()
    lg = nc.declare_dram_parameter("logits", [P, ncol], F16, isOutput=False)
    wp = nc.declare_dram_parameter("wmat", [P, 32], F16, isOutput=False)
    idp = nc.declare_dram_parameter("ident", [P, 96], F16, isOutput=False)
    op = nc.declare_dram_parameter("offs", [tw, nbatch * dcols], F32, False)
    ex = nc.declare_dram_parameter(
        "expected", [tw, nbatch * dcols], F32, isOutput=True
    )

    nbuf = min(ngrp, 4)
    with (
        nc.sbuf_tensor([P, nbuf, P1_GRP * cc], F16) as tlg,
        nc.sbuf_tensor([P, nbuf, P1_GRP * cc], F16) as te,
        nc.sbuf_tensor([P, 32], F16) as tw_,
        nc.sbuf_tensor([P, 96], F16) as tid,
        nc.sbuf_tensor([P, 2, cc], F16) as sbc,        # psum evacuation
        nc.sbuf_tensor([P, nbatch * dcols], F32) as trec,
        nc.sbuf_tensor([P, nbatch * dcols], F32) as toffs,
        nc.sbuf_tensor([P, nbatch * dcols], F32) as ebuf,
        nc.psum_tensor([P, cc], F32) as ps0,
        nc.psum_tensor([P, cc], F32) as ps1,
        nc.psum_tensor([P, 2 * ntile, 96], F16) as pt0,
        nc.psum_tensor([P, 2 * ntile, 96], F16) as pt1,
        nc.Block() as block,
        nc.semaphore("wsem") as wsem,
        nc.semaphore("lsem") as lsem,
        nc.semaphore("esem") as esem,
        nc.semaphore("msem") as msem,
        nc.semaphore("csem") as csem,
        nc.semaphore("tsem") as tsem,
        nc.semaphore("vsem") as vsem,
        nc.semaphore("osem") as osem,
    ):
        ps = [ps0, ps1]
        pt = [pt0, pt1]

        @block.sync
        def _(sync):
            sync.dma_start(out=tw_[:], in_=wp[:]).then_inc(wsem, 16)
            sync.dma_start(out=tid[:], in_=idp[:]).then_inc(wsem, 16)
            sync.dma_start(out=toffs[0:tw, :], in_=op[:]).then_inc(wsem, 16)
            for gi in range(ngrp):
                if gi >= nbuf:
                    sync.wait_ge(esem, gi - nbuf + 1)    # tlg buffer reuse
                c0 = cum[gi] - gs[gi]
                sync.dma_start(
                    out=tlg[:, gi % nbuf, 0 : gs[gi] * cc],
                    in_=lg[:, c0 * cc : cum[gi] * cc],
                ).then_inc(lsem, 16)
            sync.wait_ge(vsem, nbatch)
            sync.dma_start(out=ex[:], in_=ebuf[0:tw, :]).then_inc(osem, 16)
            sync.wait_ge(osem, 16)

        @block.scalar
        def _(scalar):
            for gi in range(ngrp):
                scalar.wait_ge(lsem, 16 * (gi + 1))
                if gi >= nbuf:
                    scalar.wait_ge(msem, cum[gi - nbuf])   # te buffer reuse
                nc.scalar.activation(
                    out=te[:, gi % nbuf, 0 : gs[gi] * cc],
                    in_=tlg[:, gi % nbuf, 0 : gs[gi] * cc],
                    func=mybir.ActivationFunctionType.Exp,
                ).then_inc(esem, 1)

        def _mm_group(tensor, gi):
            tensor.wait_ge(esem, gi + 1)
            if gi >= 2:
                tensor.wait_ge(csem, gi - 1)   # psum tile reuse
            for t in range(gs[gi]):
                nc.tensor.matmul(
                    out=ps[gi % 2][32 * t : 32 * t + 32, :],
                    lhsT=tw_[:],
                    rhs=te[:, gi % nbuf, t * cc : (t + 1) * cc],
                    start=True, stop=True,
                ).then_inc(msem, 1)

        def _tr_group(tensor, gi):
            tensor.wait_ge(csem, gi + 1)
            for k in range(ntile):
                nc.tensor.transpose(
                    out=pt[gi // 2][0:tw, (gi % 2) * ntile + k, :],
                    in_=sbc[0:96, gi % 2, k * tw : (k + 1) * tw],
                    identity=tid[0:96, :],
                ).then_inc(tsem, 1)

        @block.tensor
        def _(tensor):
            tensor.wait_ge(wsem, 32)
            _mm_group(tensor, 0)
            for gi in range(1, ngrp):
                _mm_group(tensor, gi)
                _tr_group(tensor, gi - 1)
            _tr_group(tensor, ngrp - 1)

        def _div(vector, bi, glast):
            vector.wait_ge(tsem, ntile * (glast + 1))
            pb = pt[bi][0:tw, :, :].rearrange("p a (t m) -> p a t m", m=32)
            sl = slice(bi * dcols, (bi + 1) * dcols)
            nc.vector.reciprocal(
                out=trec[0:tw, sl], in_=pb[:, :, :, 0:8]
            )
            nc.vector.tensor_tensor(
                out=ebuf[0:tw, sl], in0=pb[:, :, :, 8:16],
                in1=trec[0:tw, sl], op=mybir.AluOpType.mult,
            )
            nc.vector.tensor_tensor(
                out=ebuf[0:tw, sl], in0=ebuf[0:tw, sl],
                in1=toffs[0:tw, sl], op=mybir.AluOpType.add,
            ).then_inc(vsem, 1)

        @block.vector
        def _(vector):
            vector.wait_ge(wsem, 48)
            for gi in range(ngrp):
                vector.wait_ge(msem, cum[gi])
                if gi >= 2:
                    vector.wait_ge(tsem, ntile * (gi - 1))   # sbc reuse
                nc.vector.tensor_copy(
                    out=sbc[0:96, gi % 2, :], in_=ps[gi % 2][0:96, :]
                ).then_inc(csem, 1)
                if gi % 2 == 1:
                    _div(vector, gi // 2, gi)
            if ngrp % 2 == 1:
                _div(vector, ngrp // 2, ngrp - 1)

    return nc


# --------------------------------------------------------------------------
# phase 2: w = xg*ft, tree segment-sum -> Ax, viol stats
# --------------------------------------------------------------------------
def _build_phase2(tiers):
    """tiers: tuple of (L, Spad, Rc) per degree tier; chunk = Rc ranks.

    Per chunk: DVE multiplies xg*ft in place, GPSIMD does the big first
    halving of the slot tree, DVE finishes the tree into tax.  Stats run
    in two pieces (mid-stream + tail) and are combined."""
    global LAST_P2_ARGS
    LAST_P2_ARGS = (tiers,)
    nc = bass.Bass()
    ax_tot = sum(s for _, s, _ in tiers)
    chunks = []          # (tier_idx, chunk_idx, axbase)
    axb = 0
    for t, (L, Spad, Rc) in enumerate(tiers):
        for ch in range(Spad // Rc):
            chunks.append((t, ch, axb + ch * Rc))
        axb += Spad
    chmax = max(2 * L * Rc for L, _, Rc in tiers)
    nchunks = len(chunks)
    # stats split point: last chunk of the tier crossing 55% of ax columns
    assert len(tiers) >= 2
    axA = 0
    cA = nchunks - 1
    acc = 0
    for t, (L, Spad, Rc) in enumerate(tiers[:-1]):
        acc += Spad
        if acc >= 0.55 * ax_tot:
            axA = acc
            cA = sum(s // r for _, s, r in tiers[: t + 1]) - 1
            break
    if axA == 0:  # fall back: piece A = all tiers but the last
        axA = ax_tot - tiers[-1][1]
        cA = nchunks - tiers[-1][1] // tiers[-1][2] - 1

    xs = [
        nc.declare_dram_parameter(f"st{t}", [P, Spad * 2 * L], F16, False)
        for t, (L, Spad, Rc) in enumerate(tiers)
    ]
    bs = nc.declare_dram_parameter("bias", [P, ax_tot], F16, isOutput=False)
    out_p = nc.declare_dram_parameter("partials", [P, 4], F32, isOutput=True)

    with (
        nc.sbuf_tensor([P, NBUF2, chmax], F16) as tst,
        nc.sbuf_tensor([P, ax_tot], F16) as tax,
        nc.sbuf_tensor([P, ax_tot], F16) as tb,
        nc.sbuf_tensor([P, ax_tot], F16) as tv,
        nc.sbuf_tensor([P, 2], F16) as tm16,
        nc.sbuf_tensor([P, 2], F32) as tsum,
        nc.sbuf_tensor([P, 2], F32) as tcnt,
        nc.sbuf_tensor([P, 4], F32) as tout,
        nc.Block() as block,
        nc.semaphore("bsem") as bsem,
        nc.semaphore("pa") as pa,
        nc.semaphore("m2") as m2,
        nc.semaphore("g2") as g2,
        nc.semaphore("vs") as vs,
        nc.semaphore("fsem") as fsem,
        nc.semaphore("osem") as osem,
    ):

        @block.sync
        def _(sync):
            sync.dma_start(out=tb[:], in_=bs[:]).then_inc(bsem, 16)
            for i, (t, ch, _axb) in enumerate(chunks):
                L, Spad, Rc = tiers[t]
                sz = 2 * L * Rc
                if i >= NBUF2:
                    sync.wait_ge(vs, i - NBUF2 + 1)
                sync.dma_start(
                    out=tst[:, i % NBUF2, 0:sz],
                    in_=xs[t][:, ch * sz : (ch + 1) * sz],
                ).then_inc(pa, 16)
            sync.wait_ge(fsem, 2)
            sync.dma_start(out=out_p[:], in_=tout[:]).then_inc(osem, 16)
            sync.wait_ge(osem, 16)

        @block.gpsimd
        def _(gpsimd):
            for i, (t, ch, axb_c) in enumerate(chunks):
                L, Spad, Rc = tiers[t]
                h1 = L if L % 2 == 0 else L - 1
                hf = (h1 // 2) * Rc
                gpsimd.wait_ge(m2, i + 1)
                w = tst[:, i % NBUF2, :]
                nc.gpsimd.tensor_tensor(
                    out=w[0:P, 0:hf], in0=w[0:P, 0:hf],
                    in1=w[0:P, hf : 2 * hf], op=mybir.AluOpType.add,
                ).then_inc(g2, 1)

        def _tree_upper(i):
            t, ch, axb_c = chunks[i]
            L, Spad, Rc = tiers[t]
            w = tst[:, i % NBUF2, :]
            if L % 2 == 1:
                nc.vector.tensor_tensor(
                    out=w[0:P, 0:Rc], in0=w[0:P, 0:Rc],
                    in1=w[0:P, (L - 1) * Rc : L * Rc],
                    op=mybir.AluOpType.add,
                )
            h = (L if L % 2 == 0 else L - 1) // 2
            if h <= 1:
                nc.vector.tensor_copy(
                    out=tax[:, axb_c : axb_c + Rc], in_=w[0:P, 0:Rc]
                ).then_inc(vs, 1)
                return
            while True:
                if h % 2 == 1:
                    nc.vector.tensor_tensor(
                        out=w[0:P, 0:Rc], in0=w[0:P, 0:Rc],
                        in1=w[0:P, (h - 1) * Rc : h * Rc],
                        op=mybir.AluOpType.add,
                    )
                    h -= 1
                if h == 2:
                    nc.vector.tensor_tensor(
                        out=tax[:, axb_c : axb_c + Rc],
                        in0=w[0:P, 0:Rc], in1=w[0:P, Rc : 2 * Rc],
                        op=mybir.AluOpType.add,
                    ).then_inc(vs, 1)
                    break
                hf = (h // 2) * Rc
                nc.vector.tensor_tensor(
                    out=w[0:P, 0:hf], in0=w[0:P, 0:hf],
                    in1=w[0:P, hf : 2 * hf],
                    op=mybir.AluOpType.add,
                )
                h //= 2

        def _stats(piece, lo, hi):
            nc.vector.tensor_tensor(
                out=tv[:, lo:hi], in0=tax[:, lo:hi], in1=tb[:, lo:hi],
                op=mybir.AluOpType.subtract,
            )
            nc.vector.tensor_scalar_max(
                out=tv[:, lo:hi], in0=tv[:, lo:hi], scalar1=0.0
            )
            nc.vector.tensor_reduce(
                out=tsum[:, piece : piece + 1], in_=tv[:, lo:hi],
                axis=mybir.AxisListType.X, op=mybir.AluOpType.add,
            )
            nc.vector.tensor_reduce(
                out=tm16[:, piece : piece + 1], in_=tv[:, lo:hi],
                axis=mybir.AxisListType.X, op=mybir.AluOpType.max,
            )
            nc.vector.tensor_scalar(
                out=tv[:, lo:hi], in0=tv[:, lo:hi], scalar1=CNT_THR,
                scalar2=None, op0=mybir.AluOpType.is_gt,
            )
            nc.vector.tensor_reduce(
                out=tcnt[:, piece : piece + 1], in_=tv[:, lo:hi],
                axis=mybir.AxisListType.X, op=mybir.AluOpType.add,
            )

        @block.vector
        def _(vector):
            vector.wait_ge(bsem, 16)
            for i, (t, ch, axb_c) in enumerate(chunks):
                L, Spad, Rc = tiers[t]
                m = L * Rc
                vector.wait_ge(pa, 16 * (i + 1))
                w = tst[:, i % NBUF2, :]
                nc.vector.tensor_tensor(
                    out=w[0:P, 0:m], in0=w[0:P, 0:m], in1=w[0:P, m : 2 * m],
                    op=mybir.AluOpType.mult,
                ).then_inc(m2, 1)
                if i > 0:
                    vector.wait_ge(g2, i)
                    _tree_upper(i - 1)
                if i - 1 == cA:
                    _stats(0, 0, axA)
            vector.wait_ge(g2, nchunks)
            _tree_upper(nchunks - 1)
            if cA == nchunks - 1:
                _stats(0, 0, axA)
            _stats(1, axA, ax_tot)
            # combine the two pieces
            nc.vector.tensor_tensor(
                out=tout[:, 0:1], in0=tsum[:, 0:1], in1=tsum[:, 1:2],
                op=mybir.AluOpType.add,
            )
            nc.vector.tensor_reduce(
                out=tout[:, 1:2], in_=tm16[:],
                axis=mybir.AxisListType.X, op=mybir.AluOpType.max,
            )
            nc.vector.tensor_tensor(
                out=tout[:, 2:3], in0=tcnt[:, 0:1], in1=tcnt[:, 1:2],
                op=mybir.AluOpType.add,
            )
            nc.vector.tensor_copy(out=tout[:, 3:4], in_=tout[:, 2:3]).then_inc(
                fsem, 2
            )

    return nc


def _ceil_div(a, b):
    return -(-a // b)


# --------------------------------------------------------------------------
# host-side layout prep (index shuffling only)
# --------------------------------------------------------------------------
def _p1_rowmap(nch, cc):
    """row_of[j, bi, ti, t, g] -> packed row index (or -1 if the slot is
    junk), matching the device's e/offs column order col = bi*dcols +
    ti*24 + t*8 + g with partition j."""
    gs = _p1_groups(nch)
    ngrp = len(gs)
    nbatch = _ceil_div(ngrp, 2)
    tw = P1_TW
    ntile = cc // tw
    ncol = nch * cc
    j, bi, ti, t, g = np.meshgrid(
        np.arange(tw), np.arange(nbatch), np.arange(2 * ntile),
        np.arange(P1_GRP), np.arange(8), indexing="ij",
    )
    gi = bi * 2 + ti // ntile
    k4 = ti % ntile
    valid = (gi < ngrp) & (t < np.asarray(gs + [0])[np.minimum(gi, ngrp)])
    chunk = np.cumsum([0] + gs)[np.minimum(gi, ngrp - 1)] + t
    row = g * ncol + chunk * cc + k4 * tw + j
    row = np.where(valid, row, -1)
    return row, ncol


def _prep_phase1(logits, offsets):
    """Pack per-core class-major logits grids + offsets; return arrays."""
    ns, ccls = logits.shape
    assert ccls == 16
    rows_core = _ceil_div(ns, NCORES)
    ncol_need = _ceil_div(rows_core, 8)
    cc = P1_CC
    nch = _ceil_div(ncol_need, cc)
    ncol = nch * cc
    rows_cap = 8 * ncol
    tw = P1_TW

    # weight block: cols 0..7 ones per group, 8..15 class values, 16..31 zero
    W = np.zeros((P, 32), dtype=np.float16)
    pidx = np.arange(P)
    g = pidx // 16
    c = pidx % 16
    W[pidx, g] = 1.0
    W[pidx, 8 + g] = c.astype(np.float16)
    ident = np.zeros((P, 96), dtype=np.float16)
    ident[np.arange(96), np.arange(96)] = 1.0

    row_of, _ = _p1_rowmap(nch, cc)
    flat = row_of.reshape(tw, -1)

    lgs, offs_packed = [], []
    for core in range(NCORES):
        lo, hi = core * rows_core, min((core + 1) * rows_core, ns)
        lgp = np.zeros((rows_cap, ccls), dtype=np.float16)
        lgp[: hi - lo] = logits[lo:hi].astype(np.float16)
        # partition p = g*16 + cls, column j; row r = g*ncol + j
        lgs.append(
            np.ascontiguousarray(
                lgp.reshape(8, ncol, ccls).transpose(0, 2, 1).reshape(P, ncol)
            )
        )
        ofp = np.zeros(rows_cap + 1, dtype=np.float32)
        ofp[: hi - lo] = offsets[lo:hi]
        offs_packed.append(np.ascontiguousarray(ofp[flat]))
    return (nch, cc), W, ident, lgs, offs_packed, rows_core, rows_cap


def _unpack_expected(e_packed, nch, cc, rows_cap, nrows):
    row_of, _ = _p1_rowmap(nch, cc)
    flat = row_of.reshape(-1)
    ok = flat >= 0
    out = np.zeros(rows_cap, dtype=np.float32)
    out[flat[ok]] = e_packed.reshape(-1)[ok]
    return out[:nrows]


def _prep_phase2(con, var, feat, bias, n_con):
    """Sort edges, tier by degree, slot-major layout. Returns metadata +
    per-tier (core-major) index/feature arrays; xg filled later."""
    ne = con.shape[0]
    deg = np.bincount(con, minlength=n_con)
    order = np.argsort(con, kind="stable")
    run_start = np.zeros(n_con + 1, dtype=np.int64)
    np.cumsum(deg, out=run_start[1:])
    con_sorted = con[order]
    off_in_run = np.arange(ne, dtype=np.int64) - run_start[con_sorted]
    var_sorted = var[order]
    feat_sorted = feat[order]

    maxdeg = int(deg.max()) if ne else 1
    cand = list(range(16, 68, 4))
    if maxdeg > cand[-1]:
        cand.append(_ceil_div(maxdeg, 4) * 4)
    cand = np.asarray(cand, dtype=np.int64)
    t_cand = np.searchsorted(cand, deg, side="left")
    cnt = np.bincount(t_cand, minlength=len(cand))
    # merge small tiers upward into the next stride
    keep = []
    acc = 0
    remap = np.zeros(len(cand), dtype=np.int64)
    for si in range(len(cand)):
        acc += cnt[si]
        remap[si] = len(keep)
        if (acc >= MIN_TIER) or (si == len(cand) - 1 and acc > 0):
            keep.append(int(cand[si]))
            acc = 0
    t_of_seg = remap[t_cand]

    raw = []
    for t, L in enumerate(keep):
        segs = np.nonzero(t_of_seg == t)[0]
        n_t = segs.shape[0]
        if n_t == 0:
            continue
        S_t = _ceil_div(n_t, NBINS)
        nch_t = max(1, int(round(S_t * 2 * L / CH_TARGET)))
        nch_t = min(nch_t, S_t)
        Rc = _ceil_div(S_t, nch_t)
        Spad = nch_t * Rc
        raw.append((t, L, Spad, Rc, segs))
    # processing order = tier order: small tier first and last (short
    # pipeline fill/drain), big tiers in the middle
    order = sorted(range(len(raw)), key=lambda i: -raw[i][2] * raw[i][1])
    if len(order) >= 3:
        order = [order[-2]] + order[:-2] + [order[-1]]

    tiers = []
    tier_data = []
    axb = 0
    dense = np.full(len(keep), -1, dtype=np.int64)
    for i in order:
        t, L, Spad, Rc, segs = raw[i]
        dense[t] = len(tiers)
        k_of_con = np.full(n_con, -1, dtype=np.int64)
        k_of_con[segs] = np.arange(segs.shape[0])
        tiers.append((L, Spad, Rc))
        tier_data.append((segs, k_of_con, axb))
        axb += Spad
    t_of_seg = dense[t_of_seg]

    ax_tot = axb
    bias_arr = np.full((NCORES, P, ax_tot), BIG_BIAS, dtype=np.float16)
    for (L, Spad, Rc), (segs, k_of_con, axb) in zip(tiers, tier_data):
        k = k_of_con[segs]
        bb = k % NBINS
        r = k // NBINS
        bias_arr[bb // P, bb % P, axb + r] = bias[segs].astype(np.float16)

    return (
        tiers,
        tier_data,
        ax_tot,
        bias_arr,
        con_sorted,
        off_in_run,
        var_sorted,
        feat_sorted,
        t_of_seg,
    )


def _fill_streams(tiers, tier_data, t_of_seg, con_sorted, off_in_run,
                  var_sorted, feat_sorted, x16):
    """Build per-tier interleaved (xg, ft) fp16 streams, slot-major."""
    e_tier = t_of_seg[con_sorted]
    streams = []
    for t, ((L, Spad, Rc), (segs, k_of_con, axb)) in enumerate(
        zip(tiers, tier_data)
    ):
        sel = np.nonzero(e_tier == t)[0]
        cs = con_sorted[sel]
        slot = off_in_run[sel]
        k = k_of_con[cs]
        b = k % NBINS
        r = k // NBINS
        core = b // P
        part = b % P
        ch = r // Rc
        rin = r % Rc
        base = ch * (2 * L * Rc)
        col_x = base + slot * Rc + rin
        col_f = base + (L + slot) * Rc + rin
        width = Spad * 2 * L
        arr = np.zeros(NCORES * P * width, dtype=np.float16)
        flat_base = (core * P + part) * width
        arr[flat_base + col_x] = x16[var_sorted[sel]]
        arr[flat_base + col_f] = feat_sorted[sel].astype(np.float16)
        streams.append(arr.reshape(NCORES, P, width))
    return streams


# --------------------------------------------------------------------------
def kernel(**inputs) -> tuple:
    prob_bin = np.asarray(inputs["prob_bin"], dtype=np.float32)
    logits = np.asarray(inputs["logits_int_small"], dtype=np.float32)
    offsets = np.asarray(inputs["int_small_offsets"], dtype=np.float32)
    pred_l = np.asarray(inputs["pred_int_large"], dtype=np.float32)
    feat = np.asarray(inputs["edge_features"], dtype=np.float32).reshape(-1)
    cfeat = np.asarray(inputs["constraint_features"], dtype=np.float32)
    vfeat = np.asarray(inputs["variable_features"], dtype=np.float32)
    idx_bin = np.asarray(inputs["idx_bin"], dtype=np.int64)
    idx_s = np.asarray(inputs["idx_int_small"], dtype=np.int64)
    idx_l = np.asarray(inputs["idx_int_large"], dtype=np.int64)
    var_types = np.asarray(inputs["var_types"], dtype=np.int64)
    ei = np.asarray(inputs["edge_indices"], dtype=np.int64)
    n_vars = int(inputs["n_vars"])

    n_con = cfeat.shape[0]
    ns = logits.shape[0]
    bias = np.ascontiguousarray(cfeat[:, BIAS_COL])
    lp_vals = np.ascontiguousarray(vfeat[:, LP_SOL_COL])

    # ---------------- launch 1 ----------------
    (nch, cc), W, ident, lgs, offs_packed, rows_core, rows_cap = _prep_phase1(
        logits, offsets
    )
    nc1 = _build_phase1((nch, cc))
    in1 = [
        {"logits": lgs[c], "wmat": W, "ident": ident, "offs": offs_packed[c]}
        for c in range(NCORES)
    ]
    res1 = run_bass_kernel_spmd(nc1, in1, list(range(NCORES)))
    expected = np.concatenate(
        [
            _unpack_expected(
                res1.results[c]["expected"], nch, cc, rows_cap, rows_core
            )
            for c in range(NCORES)
        ]
    )[:ns]

    # ---------------- host: assemble x ----------------
    xfull = np.zeros(n_vars, dtype=np.float32)
    xfull[idx_bin] = prob_bin[:, 0]
    xfull[idx_s] = expected
    xfull[idx_l] = pred_l[:, 0]
    xfull = np.where(var_types == 0, lp_vals, xfull)
    x16 = xfull.astype(np.float16)

    # ---------------- launch 2 ----------------
    (
        tiers, tier_data, ax_tot, bias_arr, con_sorted, off_in_run,
        var_sorted, feat_sorted, t_of_seg,
    ) = _prep_phase2(ei[0], ei[1], feat, bias, n_con)
    streams = _fill_streams(
        tiers, tier_data, t_of_seg, con_sorted, off_in_run, var_sorted,
        feat_sorted, x16,
    )
    nc2 = _build_phase2(tuple(tiers))
    in2 = []
    for c in range(NCORES):
        m = {f"st{t}": streams[t][c] for t in range(len(tiers))}
        m["bias"] = bias_arr[c]
        in2.append(m)
    res2 = run_bass_kernel_spmd(nc2, in2, list(range(NCORES)))

    parts = np.stack([res2.results[c]["partials"] for c in range(NCORES)])
    vsum = parts[:, :, 0].astype(np.float64).sum()
    vmax = np.float32(parts[:, :, 1].max())
    vcnt = np.int64(round(float(parts[:, :, 2].astype(np.float64).sum())))
    mean_viol = np.float32(vsum / np.float64(n_con))
    penalty = np.float32(
        np.float32(LAMBDA_MEAN) * mean_viol + np.float32(LAMBDA_MAX) * vmax
    )
    return penalty, mean_viol, vmax, vcnt


# revision 40
# speedup vs baseline: 1.9396x; 1.0581x over previous
"""Trainium2 Bass kernel for ConstraintViolationLoss (GNN message passing).

Two launches on 8 NeuronCores (SPMD), fp16 data streams:

  Launch 1 (softmax expected-value head): logits are laid out class-on-
  partition ([128, ncol] tiles, partition p = 16*g + c holding class c of
  row-group g), ACT computes exp in fp16, and ONE PE matmul against a
  constant [128, 16] weight block produces both softmax sums per row
  (denominator via ones-blocks, numerator via class-value blocks) in PSUM.
  A DRAM bounce regroups the [16, cc] PSUM tile to [128, *] so the DVE
  divide/add runs with all partitions active.

  Host then assembles x (index scatter only), gathers x along the sorted
  edge list, and lays edge (x, feature) pairs out slot-major per
  constraint-degree tier so the per-constraint segment sum becomes a
  binary tree of contiguous fp16 tensor_tensor adds (2x DVE mode).

  Launch 2: per chunk w = xg * ft (fp16, in place), tree-reduce to Ax,
  then one stats pass: viol = relu(Ax - bias), sum / max / count.
"""

import sys

sys.path.insert(0, "/opt/trn_rl_repo")

import numpy as np

import concourse.bass as bass
import concourse.mybir as mybir
from concourse.bass_utils import run_bass_kernel_spmd

P = 128
NCORES = 8
NBINS = P * NCORES
LAMBDA_MEAN, LAMBDA_MAX = 1.0, 0.1
BIAS_COL = 1
LP_SOL_COL = 8
BIG_BIAS = 60000.0          # fp16-safe "never violated" bias for padding segs
CNT_THR = 1e-6
F16 = mybir.dt.float16
F32 = mybir.dt.float32

# phase-1 geometry
P1_CC = 480                 # columns per chunk (rows = 8 per column)
P1_GRP = 3                  # chunks per PSUM group (row offsets 0/32/64)
P1_TW = 120                 # transpose tile width (out partitions)
# phase-2 chunking
CH_TARGET = 3072            # target stream elems / partition / chunk
NBUF2 = 4                   # stream buffers in phase 2
MIN_TIER = 6 * NBINS        # merge degree tiers smaller than this

# most recent build params, for the test harness
LAST_ROWS_PP = None
LAST_P2_ARGS = None


# --------------------------------------------------------------------------
# phase 1: expected = (softmax(logits) @ [0..C)) + offsets
# --------------------------------------------------------------------------
def _p1_groups(nch):
    gs = []
    left = nch
    while left > 0:
        gs.append(min(P1_GRP, left))
        left -= P1_GRP
    return gs


def _build_phase1(params):
    """Chunk c of group gi writes its [32, cc] matmul output (rows: D_g at
    32t+g, N_g at 32t+8+g) into a shared [128, cc] PSUM tile.  DVE
    evacuates each group to SBUF as fp16, PE transposes 120-column tiles so
    D and N land on the same partition (different free offsets), and DVE
    divides batches of two groups at once."""
    global LAST_ROWS_PP
    LAST_ROWS_PP = params
    nch, cc = params
    gs = _p1_groups(nch)
    ngrp = len(gs)
    cum = np.cumsum(gs).tolist()
    nbatch = _ceil_div(ngrp, 2)
    tw = P1_TW
    ntile = cc // tw            # transpose tiles per group
    dcols = ntile * 2 * 24      # e-columns per group-pair batch
    ncol = nch * cc

    nc = bass.Bass()
    lg = nc.declare_dram_parameter("logits", [P, ncol], F16, isOutput=False)
    wp = nc.declare_dram_parameter("wmat", [P, 32], F16, isOutput=False)
    idp = nc.declare_dram_parameter("ident", [P, 96], F16, isOutput=False)
    op = nc.declare_dram_parameter("offs", [tw, nbatch * dcols], F32, False)
    ex = nc.declare_dram_parameter(
        "expected", [tw, nbatch * dcols], F32, isOutput=True
    )

    nbuf = min(ngrp, 4)
    gdc = ntile * 24            # e-columns per group
    from contextlib import ExitStack

    with ExitStack() as st:
        ec = st.enter_context
        tlg = ec(nc.sbuf_tensor([P, nbuf, P1_GRP * cc], F16))
        te = ec(nc.sbuf_tensor([P, nbuf, P1_GRP * cc], F16))
        tw_ = ec(nc.sbuf_tensor([P, 32], F16))
        tid = ec(nc.sbuf_tensor([P, 96], F16))
        twu = ec(nc.sbuf_tensor([P, 32], F16))         # PE warmup source
        sbc = ec(nc.sbuf_tensor([P, 2, cc], F16))      # psum evacuation
        trec = ec(nc.sbuf_tensor([P, gdc], F32))
        toffs = ec(nc.sbuf_tensor([P, ngrp * gdc], F32))
        ebuf = ec(nc.sbuf_tensor([P, ngrp * gdc], F32))
        ps = [
            ec(nc.psum_tensor(f"ps{i}", [P, cc], F32)) for i in range(2)
        ]
        pt = [
            ec(nc.psum_tensor(f"pt{i}", [P, ntile, 96], F16))
            for i in range(2)
        ]
        pwu = ec(nc.psum_tensor("pwu", [32, 32], F32))
        block = ec(nc.Block())
        wsem = ec(nc.semaphore("wsem"))
        usem = ec(nc.semaphore("usem"))
        lsem = ec(nc.semaphore("lsem"))
        esem = ec(nc.semaphore("esem"))
        msem = ec(nc.semaphore("msem"))
        csem = ec(nc.semaphore("csem"))
        tsem = ec(nc.semaphore("tsem"))
        vsem = ec(nc.semaphore("vsem"))
        osem = ec(nc.semaphore("osem"))

        @block.sync
        def _(sync):
            for gi in range(ngrp):
                if gi >= nbuf:
                    sync.wait_ge(esem, gi - nbuf + 1)    # tlg buffer reuse
                c0 = cum[gi] - gs[gi]
                sync.dma_start(
                    out=tlg[:, gi % nbuf, 0 : gs[gi] * cc],
                    in_=lg[:, c0 * cc : cum[gi] * cc],
                ).then_inc(lsem, 16)
            half_b = nbatch // 2
            sync.wait_ge(vsem, 2 * half_b)
            sync.dma_start(
                out=ex[:, 0 : 2 * half_b * gdc],
                in_=ebuf[0:tw, 0 : 2 * half_b * gdc],
            ).then_inc(osem, 16)
            sync.wait_ge(vsem, ngrp)
            sync.dma_start(
                out=ex[:, 2 * half_b * gdc :],
                in_=ebuf[0:tw, 2 * half_b * gdc :],
            ).then_inc(osem, 16)
            sync.wait_ge(osem, 32)

        @block.scalar
        def _(scalar):
            scalar.dma_start(out=tw_[:], in_=wp[:]).then_inc(wsem, 16)
            scalar.dma_start(out=tid[:], in_=idp[:]).then_inc(wsem, 16)
            scalar.dma_start(out=toffs[0:tw, :], in_=op[:]).then_inc(wsem, 16)
            for gi in range(ngrp):
                scalar.wait_ge(lsem, 16 * (gi + 1))
                if gi >= nbuf:
                    scalar.wait_ge(msem, cum[gi - nbuf])   # te buffer reuse
                nc.scalar.activation(
                    out=te[:, gi % nbuf, 0 : gs[gi] * cc],
                    in_=tlg[:, gi % nbuf, 0 : gs[gi] * cc],
                    func=mybir.ActivationFunctionType.Exp,
                ).then_inc(esem, 1)

        @block.gpsimd
        def _(gpsimd):
            gpsimd.memset(twu[:], 0.0)
            gpsimd.drain().then_inc(usem, 1)

        def _mm_group(tensor, gi):
            tensor.wait_ge(esem, gi + 1)
            if gi >= 2:
                tensor.wait_ge(csem, gi - 1)   # psum tile reuse
            for t in range(gs[gi]):
                nc.tensor.matmul(
                    out=ps[gi % 2][32 * t : 32 * t + 32, :],
                    lhsT=tw_[:],
                    rhs=te[:, gi % nbuf, t * cc : (t + 1) * cc],
                    start=True, stop=True,
                ).then_inc(msem, 1)

        def _tr_group(tensor, gi):
            tensor.wait_ge(csem, gi + 1)
            if gi >= 2:
                tensor.wait_ge(vsem, gi - 1)   # psumT tile reuse
            for k in range(ntile):
                nc.tensor.transpose(
                    out=pt[gi % 2][0:tw, k, :],
                    in_=sbc[0:96, gi % 2, k * tw : (k + 1) * tw],
                    identity=tid[0:96, :],
                ).then_inc(tsem, 1)

        @block.tensor
        def _(tensor):
            # keep the PE busy early so it is at full p-state for real work
            tensor.wait_ge(usem, 1)
            for _ in range(10):
                nc.tensor.matmul(
                    out=pwu[:, :], lhsT=twu[:], rhs=twu[:],
                    start=True, stop=True,
                )
            tensor.wait_ge(wsem, 32)
            _mm_group(tensor, 0)
            for gi in range(1, ngrp):
                _mm_group(tensor, gi)
                _tr_group(tensor, gi - 1)
            _tr_group(tensor, ngrp - 1)

        def _div(vector, gi):
            vector.wait_ge(tsem, ntile * (gi + 1))
            pb = pt[gi % 2][0:tw, :, :].rearrange("p a (t m) -> p a t m", m=32)
            sl = slice(gi * gdc, (gi + 1) * gdc)
            nc.vector.reciprocal(out=trec[0:tw, :], in_=pb[:, :, :, 0:8])
            nc.vector.tensor_tensor(
                out=ebuf[0:tw, sl], in0=pb[:, :, :, 8:16],
                in1=trec[0:tw, :], op=mybir.AluOpType.mult,
            )
            nc.vector.tensor_tensor(
                out=ebuf[0:tw, sl], in0=ebuf[0:tw, sl],
                in1=toffs[0:tw, sl], op=mybir.AluOpType.add,
            ).then_inc(vsem, 1)

        @block.vector
        def _(vector):
            vector.wait_ge(wsem, 48)
            for gi in range(ngrp):
                vector.wait_ge(msem, cum[gi])
                if gi >= 2:
                    vector.wait_ge(tsem, ntile * (gi - 1))   # sbc reuse
                nc.vector.tensor_copy(
                    out=sbc[0:96, gi % 2, :], in_=ps[gi % 2][0:96, :]
                ).then_inc(csem, 1)
                if gi >= 1:
                    _div(vector, gi - 1)
            _div(vector, ngrp - 1)

    return nc


# --------------------------------------------------------------------------
# phase 2: w = xg*ft, tree segment-sum -> Ax, viol stats
# --------------------------------------------------------------------------
def _build_phase2(tiers):
    """tiers: tuple of (L, Spad, Rc) per degree tier; chunk = Rc ranks.

    Per chunk: DVE multiplies xg*ft in place, GPSIMD does the big first
    halving of the slot tree, DVE finishes the tree into tax.  Stats run
    in two pieces (mid-stream + tail) and are combined."""
    global LAST_P2_ARGS
    LAST_P2_ARGS = (tiers,)
    nc = bass.Bass()
    ax_tot = sum(s for _, s, _ in tiers)
    chunks = []          # (tier_idx, chunk_idx, axbase)
    axb = 0
    for t, (L, Spad, Rc) in enumerate(tiers):
        for ch in range(Spad // Rc):
            chunks.append((t, ch, axb + ch * Rc))
        axb += Spad
    chmax = max(2 * L * Rc for L, _, Rc in tiers)
    nchunks = len(chunks)
    # stats split point: last chunk of the tier crossing 55% of ax columns
    assert len(tiers) >= 2
    axA = 0
    cA = nchunks - 1
    acc = 0
    for t, (L, Spad, Rc) in enumerate(tiers[:-1]):
        acc += Spad
        if acc >= 0.62 * ax_tot:
            axA = acc
            cA = sum(s // r for _, s, r in tiers[: t + 1]) - 1
            break
    if axA == 0:  # fall back: piece A = all tiers but the last
        axA = ax_tot - tiers[-1][1]
        cA = nchunks - tiers[-1][1] // tiers[-1][2] - 1

    xs = [
        nc.declare_dram_parameter(f"st{t}", [P, Spad * 2 * L], F16, False)
        for t, (L, Spad, Rc) in enumerate(tiers)
    ]
    bs = nc.declare_dram_parameter("bias", [P, ax_tot], F16, isOutput=False)
    out_p = nc.declare_dram_parameter("partials", [P, 4], F32, isOutput=True)

    with (
        nc.sbuf_tensor([P, NBUF2, chmax], F16) as tst,
        nc.sbuf_tensor([P, ax_tot], F16) as tax,
        nc.sbuf_tensor([P, ax_tot], F16) as tb,
        nc.sbuf_tensor([P, ax_tot], F16) as tv,
        nc.sbuf_tensor([P, 2], F16) as tm16,
        nc.sbuf_tensor([P, 2], F32) as tsum,
        nc.sbuf_tensor([P, 2], F32) as tcnt,
        nc.sbuf_tensor([P, 4], F32) as tout,
        nc.Block() as block,
        nc.semaphore("bsem") as bsem,
        nc.semaphore("pa") as pa,
        nc.semaphore("m2") as m2,
        nc.semaphore("g2") as g2,
        nc.semaphore("vs") as vs,
        nc.semaphore("fsem") as fsem,
        nc.semaphore("osem") as osem,
    ):

        def _l1_split(i):
            t, ch, axb_c = chunks[i]
            L, Spad, Rc = tiers[t]
            h1 = L if L % 2 == 0 else L - 1
            hf = (h1 // 2) * Rc
            xc = min(hf, (int(0.17 * hf) + 7) // 8 * 8)
            return hf, xc

        @block.sync
        def _(sync):
            for i, (t, ch, _axb) in enumerate(chunks):
                L, Spad, Rc = tiers[t]
                sz = 2 * L * Rc
                if i >= NBUF2:
                    sync.wait_ge(vs, i - NBUF2 + 1)
                sync.dma_start(
                    out=tst[:, i % NBUF2, 0:sz],
                    in_=xs[t][:, ch * sz : (ch + 1) * sz],
                ).then_inc(pa, 16)
            sync.wait_ge(fsem, 2)
            sync.dma_start(out=out_p[:], in_=tout[:]).then_inc(osem, 16)
            sync.wait_ge(osem, 16)

        @block.scalar
        def _(scalar):
            scalar.dma_start(out=tb[:], in_=bs[:]).then_inc(bsem, 16)

        @block.gpsimd
        def _(gpsimd):
            for i, (t, ch, axb_c) in enumerate(chunks):
                hf, xc = _l1_split(i)
                gpsimd.wait_ge(m2, i + 1)
                w = tst[:, i % NBUF2, :]
                if xc < hf:
                    nc.gpsimd.tensor_tensor(
                        out=w[0:P, xc:hf], in0=w[0:P, xc:hf],
                        in1=w[0:P, hf + xc : 2 * hf], op=mybir.AluOpType.add,
                    ).then_inc(g2, 1)
                else:
                    gpsimd.sem_inc(g2, 1)

        def _tree_upper(i):
            # assumes the full first halving (DVE part + GPSIMD part) is done
            t, ch, axb_c = chunks[i]
            L, Spad, Rc = tiers[t]
            w = tst[:, i % NBUF2, :]
            if L % 2 == 1:
                nc.vector.tensor_tensor(
                    out=w[0:P, 0:Rc], in0=w[0:P, 0:Rc],
                    in1=w[0:P, (L - 1) * Rc : L * Rc],
                    op=mybir.AluOpType.add,
                )
            h = (L if L % 2 == 0 else L - 1) // 2
            if h <= 1:
                nc.vector.tensor_copy(
                    out=tax[:, axb_c : axb_c + Rc], in_=w[0:P, 0:Rc]
                ).then_inc(vs, 1)
                return
            while True:
                if h % 2 == 1:
                    nc.vector.tensor_tensor(
                        out=w[0:P, 0:Rc], in0=w[0:P, 0:Rc],
                        in1=w[0:P, (h - 1) * Rc : h * Rc],
                        op=mybir.AluOpType.add,
                    )
                    h -= 1
                if h == 2:
                    nc.vector.tensor_tensor(
                        out=tax[:, axb_c : axb_c + Rc],
                        in0=w[0:P, 0:Rc], in1=w[0:P, Rc : 2 * Rc],
                        op=mybir.AluOpType.add,
                    ).then_inc(vs, 1)
                    break
                hf = (h // 2) * Rc
                nc.vector.tensor_tensor(
                    out=w[0:P, 0:hf], in0=w[0:P, 0:hf],
                    in1=w[0:P, hf : 2 * hf],
                    op=mybir.AluOpType.add,
                )
                h //= 2

        def _stats(piece, lo, hi):
            nc.vector.tensor_tensor(
                out=tv[:, lo:hi], in0=tax[:, lo:hi], in1=tb[:, lo:hi],
                op=mybir.AluOpType.subtract,
            )
            nc.vector.tensor_scalar_max(
                out=tv[:, lo:hi], in0=tv[:, lo:hi], scalar1=0.0
            )
            nc.vector.tensor_reduce(
                out=tsum[:, piece : piece + 1], in_=tv[:, lo:hi],
                axis=mybir.AxisListType.X, op=mybir.AluOpType.add,
            )
            nc.vector.tensor_reduce(
                out=tm16[:, piece : piece + 1], in_=tv[:, lo:hi],
                axis=mybir.AxisListType.X, op=mybir.AluOpType.max,
            )
            nc.vector.tensor_scalar(
                out=tv[:, lo:hi], in0=tv[:, lo:hi], scalar1=CNT_THR,
                scalar2=None, op0=mybir.AluOpType.is_gt,
            )
            nc.vector.tensor_reduce(
                out=tcnt[:, piece : piece + 1], in_=tv[:, lo:hi],
                axis=mybir.AxisListType.X, op=mybir.AluOpType.add,
            )

        @block.vector
        def _(vector):
            for i, (t, ch, axb_c) in enumerate(chunks):
                L, Spad, Rc = tiers[t]
                m = L * Rc
                vector.wait_ge(pa, 16 * (i + 1))
                w = tst[:, i % NBUF2, :]
                nc.vector.tensor_tensor(
                    out=w[0:P, 0:m], in0=w[0:P, 0:m], in1=w[0:P, m : 2 * m],
                    op=mybir.AluOpType.mult,
                ).then_inc(m2, 1)
                hf, xc = _l1_split(i)
                if xc > 0:
                    nc.vector.tensor_tensor(
                        out=w[0:P, 0:xc], in0=w[0:P, 0:xc],
                        in1=w[0:P, hf : hf + xc], op=mybir.AluOpType.add,
                    )
                if i > 0:
                    vector.wait_ge(g2, i)
                    _tree_upper(i - 1)
                if i - 1 == cA:
                    vector.wait_ge(bsem, 16)
                    _stats(0, 0, axA)
            vector.wait_ge(g2, nchunks)
            _tree_upper(nchunks - 1)
            if cA == nchunks - 1:
                vector.wait_ge(bsem, 16)
                _stats(0, 0, axA)
            _stats(1, axA, ax_tot)
            # combine the two pieces
            nc.vector.tensor_tensor(
                out=tout[:, 0:1], in0=tsum[:, 0:1], in1=tsum[:, 1:2],
                op=mybir.AluOpType.add,
            )
            nc.vector.tensor_reduce(
                out=tout[:, 1:2], in_=tm16[:],
                axis=mybir.AxisListType.X, op=mybir.AluOpType.max,
            )
            nc.vector.tensor_tensor(
                out=tout[:, 2:3], in0=tcnt[:, 0:1], in1=tcnt[:, 1:2],
                op=mybir.AluOpType.add,
            )
            nc.vector.tensor_copy(out=tout[:, 3:4], in_=tout[:, 2:3]).then_inc(
                fsem, 2
            )

    return nc


def _ceil_div(a, b):
    return -(-a // b)


# --------------------------------------------------------------------------
# host-side layout prep (index shuffling only)
# --------------------------------------------------------------------------
def _p1_rowmap(nch, cc):
    """row_of[j, bi, ti, t, g] -> packed row index (or -1 if the slot is
    junk), matching the device's e/offs column order col = bi*dcols +
    ti*24 + t*8 + g with partition j."""
    gs = _p1_groups(nch)
    ngrp = len(gs)
    nbatch = _ceil_div(ngrp, 2)
    tw = P1_TW
    ntile = cc // tw
    ncol = nch * cc
    j, bi, ti, t, g = np.meshgrid(
        np.arange(tw), np.arange(nbatch), np.arange(2 * ntile),
        np.arange(P1_GRP), np.arange(8), indexing="ij",
    )
    gi = bi * 2 + ti // ntile
    k4 = ti % ntile
    valid = (gi < ngrp) & (t < np.asarray(gs + [0])[np.minimum(gi, ngrp)])
    chunk = np.cumsum([0] + gs)[np.minimum(gi, ngrp - 1)] + t
    row = g * ncol + chunk * cc + k4 * tw + j
    row = np.where(valid, row, -1)
    return row, ncol


def _prep_phase1(logits, offsets):
    """Pack per-core class-major logits grids + offsets; return arrays."""
    ns, ccls = logits.shape
    assert ccls == 16
    rows_core = _ceil_div(ns, NCORES)
    ncol_need = _ceil_div(rows_core, 8)
    cc = P1_CC
    nch = _ceil_div(ncol_need, cc)
    ncol = nch * cc
    rows_cap = 8 * ncol
    tw = P1_TW

    # weight block: cols 0..7 ones per group, 8..15 class values, 16..31 zero
    W = np.zeros((P, 32), dtype=np.float16)
    pidx = np.arange(P)
    g = pidx // 16
    c = pidx % 16
    W[pidx, g] = 1.0
    W[pidx, 8 + g] = c.astype(np.float16)
    ident = np.zeros((P, 96), dtype=np.float16)
    ident[np.arange(96), np.arange(96)] = 1.0

    row_of, _ = _p1_rowmap(nch, cc)
    flat = row_of.reshape(tw, -1)

    lgs, offs_packed = [], []
    for core in range(NCORES):
        lo, hi = core * rows_core, min((core + 1) * rows_core, ns)
        lgp = np.zeros((rows_cap, ccls), dtype=np.float16)
        lgp[: hi - lo] = logits[lo:hi].astype(np.float16)
        # partition p = g*16 + cls, column j; row r = g*ncol + j
        lgs.append(
            np.ascontiguousarray(
                lgp.reshape(8, ncol, ccls).transpose(0, 2, 1).reshape(P, ncol)
            )
        )
        ofp = np.zeros(rows_cap + 1, dtype=np.float32)
        ofp[: hi - lo] = offsets[lo:hi]
        offs_packed.append(np.ascontiguousarray(ofp[flat]))
    return (nch, cc), W, ident, lgs, offs_packed, rows_core, rows_cap


def _unpack_expected(e_packed, nch, cc, rows_cap, nrows):
    row_of, _ = _p1_rowmap(nch, cc)
    flat = row_of.reshape(-1)
    ok = flat >= 0
    out = np.zeros(rows_cap, dtype=np.float32)
    out[flat[ok]] = e_packed.reshape(-1)[ok]
    return out[:nrows]


def _prep_phase2(con, var, feat, bias, n_con):
    """Sort edges, tier by degree, slot-major layout. Returns metadata +
    per-tier (core-major) index/feature arrays; xg filled later."""
    ne = con.shape[0]
    deg = np.bincount(con, minlength=n_con)
    order = np.argsort(con, kind="stable")
    run_start = np.zeros(n_con + 1, dtype=np.int64)
    np.cumsum(deg, out=run_start[1:])
    con_sorted = con[order]
    off_in_run = np.arange(ne, dtype=np.int64) - run_start[con_sorted]
    var_sorted = var[order]
    feat_sorted = feat[order]

    maxdeg = int(deg.max()) if ne else 1
    cand = list(range(16, 68, 4))
    if maxdeg > cand[-1]:
        cand.append(_ceil_div(maxdeg, 4) * 4)
    cand = np.asarray(cand, dtype=np.int64)
    t_cand = np.searchsorted(cand, deg, side="left")
    cnt = np.bincount(t_cand, minlength=len(cand))
    # merge small tiers upward into the next stride
    keep = []
    acc = 0
    remap = np.zeros(len(cand), dtype=np.int64)
    for si in range(len(cand)):
        acc += cnt[si]
        remap[si] = len(keep)
        if (acc >= MIN_TIER) or (si == len(cand) - 1 and acc > 0):
            keep.append(int(cand[si]))
            acc = 0
    t_of_seg = remap[t_cand]

    raw = []
    for t, L in enumerate(keep):
        segs = np.nonzero(t_of_seg == t)[0]
        n_t = segs.shape[0]
        if n_t == 0:
            continue
        S_t = _ceil_div(n_t, NBINS)
        nch_t = max(1, int(round(S_t * 2 * L / CH_TARGET)))
        nch_t = min(nch_t, S_t)
        Rc = _ceil_div(S_t, nch_t)
        Spad = nch_t * Rc
        raw.append((t, L, Spad, Rc, segs))
    # processing order = tier order: small tier first and last (short
    # pipeline fill/drain), big tiers in the middle
    order = sorted(range(len(raw)), key=lambda i: -raw[i][2] * raw[i][1])
    if len(order) >= 3:
        order = [order[-2]] + order[:-2] + [order[-1]]

    tiers = []
    tier_data = []
    axb = 0
    dense = np.full(len(keep), -1, dtype=np.int64)
    for i in order:
        t, L, Spad, Rc, segs = raw[i]
        dense[t] = len(tiers)
        k_of_con = np.full(n_con, -1, dtype=np.int64)
        k_of_con[segs] = np.arange(segs.shape[0])
        tiers.append((L, Spad, Rc))
        tier_data.append((segs, k_of_con, axb))
        axb += Spad
    t_of_seg = dense[t_of_seg]

    ax_tot = axb
    bias_arr = np.full((NCORES, P, ax_tot), BIG_BIAS, dtype=np.float16)
    for (L, Spad, Rc), (segs, k_of_con, axb) in zip(tiers, tier_data):
        k = k_of_con[segs]
        bb = k % NBINS
        r = k // NBINS
        bias_arr[bb // P, bb % P, axb + r] = bias[segs].astype(np.float16)

    return (
        tiers,
        tier_data,
        ax_tot,
        bias_arr,
        con_sorted,
        off_in_run,
        var_sorted,
        feat_sorted,
        t_of_seg,
    )


def _fill_streams(tiers, tier_data, t_of_seg, con_sorted, off_in_run,
                  var_sorted, feat_sorted, x16):
    """Build per-tier interleaved (xg, ft) fp16 streams, slot-major."""
    e_tier = t_of_seg[con_sorted]
    streams = []
    for t, ((L, Spad, Rc), (segs, k_of_con, axb)) in enumerate(
        zip(tiers, tier_data)
    ):
        sel = np.nonzero(e_tier == t)[0]
        cs = con_sorted[sel]
        slot = off_in_run[sel]
        k = k_of_con[cs]
        b = k % NBINS
        r = k // NBINS
        core = b // P
        part = b % P
        ch = r // Rc
        rin = r % Rc
        base = ch * (2 * L * Rc)
        col_x = base + slot * Rc + rin
        col_f = base + (L + slot) * Rc + rin
        width = Spad * 2 * L
        arr = np.zeros(NCORES * P * width, dtype=np.float16)
        flat_base = (core * P + part) * width
        arr[flat_base + col_x] = x16[var_sorted[sel]]
        arr[flat_base + col_f] = feat_sorted[sel].astype(np.float16)
        streams.append(arr.reshape(NCORES, P, width))
    return streams


# --------------------------------------------------------------------------
def kernel(**inputs) -> tuple:
    prob_bin = np.asarray(inputs["prob_bin"], dtype=np.float32)
    logits = np.asarray(inputs["logits_int_small"], dtype=np.float32)
    offsets = np.asarray(inputs["int_small_offsets"], dtype=np.float32)
    pred_l = np.asarray(inputs["pred_int_large"], dtype=np.float32)
    feat = np.asarray(inputs["edge_features"], dtype=np.float32).reshape(-1)
    cfeat = np.asarray(inputs["constraint_features"], dtype=np.float32)
    vfeat = np.asarray(inputs["variable_features"], dtype=np.float32)
    idx_bin = np.asarray(inputs["idx_bin"], dtype=np.int64)
    idx_s = np.asarray(inputs["idx_int_small"], dtype=np.int64)
    idx_l = np.asarray(inputs["idx_int_large"], dtype=np.int64)
    var_types = np.asarray(inputs["var_types"], dtype=np.int64)
    ei = np.asarray(inputs["edge_indices"], dtype=np.int64)
    n_vars = int(inputs["n_vars"])

    n_con = cfeat.shape[0]
    ns = logits.shape[0]
    bias = np.ascontiguousarray(cfeat[:, BIAS_COL])
    lp_vals = np.ascontiguousarray(vfeat[:, LP_SOL_COL])

    # ---------------- launch 1 ----------------
    (nch, cc), W, ident, lgs, offs_packed, rows_core, rows_cap = _prep_phase1(
        logits, offsets
    )
    nc1 = _build_phase1((nch, cc))
    in1 = [
        {"logits": lgs[c], "wmat": W, "ident": ident, "offs": offs_packed[c]}
        for c in range(NCORES)
    ]
    res1 = run_bass_kernel_spmd(nc1, in1, list(range(NCORES)))
    expected = np.concatenate(
        [
            _unpack_expected(
                res1.results[c]["expected"], nch, cc, rows_cap, rows_core
            )
            for c in range(NCORES)
        ]
    )[:ns]

    # ---------------- host: assemble x ----------------
    xfull = np.zeros(n_vars, dtype=np.float32)
    xfull[idx_bin] = prob_bin[:, 0]
    xfull[idx_s] = expected
    xfull[idx_l] = pred_l[:, 0]
    xfull = np.where(var_types == 0, lp_vals, xfull)
    x16 = xfull.astype(np.float16)

    # ---------------- launch 2 ----------------
    (
        tiers, tier_data, ax_tot, bias_arr, con_sorted, off_in_run,
        var_sorted, feat_sorted, t_of_seg,
    ) = _prep_phase2(ei[0], ei[1], feat, bias, n_con)
    streams = _fill_streams(
        tiers, tier_data, t_of_seg, con_sorted, off_in_run, var_sorted,
        feat_sorted, x16,
    )
    nc2 = _build_phase2(tuple(tiers))
    in2 = []
    for c in range(NCORES):
        m = {f"st{t}": streams[t][c] for t in range(len(tiers))}
        m["bias"] = bias_arr[c]
        in2.append(m)
    res2 = run_bass_kernel_spmd(nc2, in2, list(range(NCORES)))

    parts = np.stack([res2.results[c]["partials"] for c in range(NCORES)])
    vsum = parts[:, :, 0].astype(np.float64).sum()
    vmax = np.float32(parts[:, :, 1].max())
    vcnt = np.int64(round(float(parts[:, :, 2].astype(np.float64).sum())))
    mean_viol = np.float32(vsum / np.float64(n_con))
    penalty = np.float32(
        np.float32(LAMBDA_MEAN) * mean_viol + np.float32(LAMBDA_MAX) * vmax
    )
    return penalty, mean_viol, vmax, vcnt


# revision 49
# speedup vs baseline: 1.9949x; 1.0285x over previous
"""Trainium2 Bass kernel for ConstraintViolationLoss (GNN message passing).

Two launches on 8 NeuronCores (SPMD), fp16 data streams:

  Launch 1 (softmax expected-value head): logits are laid out class-on-
  partition ([128, ncol] tiles, partition p = 16*g + c holding class c of
  row-group g), ACT computes exp in fp16, and ONE PE matmul against a
  constant [128, 16] weight block produces both softmax sums per row
  (denominator via ones-blocks, numerator via class-value blocks) in PSUM.
  A DRAM bounce regroups the [16, cc] PSUM tile to [128, *] so the DVE
  divide/add runs with all partitions active.

  Host then assembles x (index scatter only), gathers x along the sorted
  edge list, and lays edge (x, feature) pairs out slot-major per
  constraint-degree tier so the per-constraint segment sum becomes a
  binary tree of contiguous fp16 tensor_tensor adds (2x DVE mode).

  Launch 2: per chunk w = xg * ft (fp16, in place), tree-reduce to Ax,
  then one stats pass: viol = relu(Ax - bias), sum / max / count.
"""

import sys

sys.path.insert(0, "/opt/trn_rl_repo")

import numpy as np

import concourse.bass as bass
import concourse.mybir as mybir
from concourse.bass_utils import run_bass_kernel_spmd

P = 128
NCORES = 8
NBINS = P * NCORES
LAMBDA_MEAN, LAMBDA_MAX = 1.0, 0.1
BIAS_COL = 1
LP_SOL_COL = 8
BIG_BIAS = 60000.0          # fp16-safe "never violated" bias for padding segs
CNT_THR = 1e-6
F16 = mybir.dt.float16
F32 = mybir.dt.float32

# phase-1 geometry
P1_CC = 480                 # columns per chunk (rows = 8 per column)
P1_GRP = 3                  # chunks per PSUM group (row offsets 0/32/64)
P1_TW = 120                 # transpose tile width (out partitions)
# phase-2 chunking
CH_TARGET = 3072            # target stream elems / partition / chunk
NBUF2 = 6                   # stream buffers in phase 2
MIN_TIER = 6 * NBINS        # merge degree tiers smaller than this

# most recent build params, for the test harness
LAST_ROWS_PP = None
LAST_P2_ARGS = None


# --------------------------------------------------------------------------
# phase 1: expected = (softmax(logits) @ [0..C)) + offsets
# --------------------------------------------------------------------------
def _p1_groups(nch):
    gs = []
    left = nch
    while left > 0:
        gs.append(min(P1_GRP, left))
        left -= P1_GRP
    return gs


def _build_phase1(params):
    """Chunk c of group gi writes its [32, cc] matmul output (rows: D_g at
    32t+g, N_g at 32t+8+g) into a shared [128, cc] PSUM tile.  DVE
    evacuates each group to SBUF as fp16, PE transposes 120-column tiles so
    D and N land on the same partition (different free offsets), and DVE
    divides batches of two groups at once."""
    global LAST_ROWS_PP
    LAST_ROWS_PP = params
    nch, cc = params
    gs = _p1_groups(nch)
    ngrp = len(gs)
    cum = np.cumsum(gs).tolist()
    nbatch = _ceil_div(ngrp, 2)
    tw = P1_TW
    ntile = cc // tw            # transpose tiles per group
    dcols = ntile * 2 * 24      # e-columns per group-pair batch
    ncol = nch * cc

    nc = bass.Bass()
    lg = nc.declare_dram_parameter("logits", [P, ncol], F16, isOutput=False)
    wp = nc.declare_dram_parameter("wmat", [P, 32], F16, isOutput=False)
    idp = nc.declare_dram_parameter("ident", [P, 96], F16, isOutput=False)
    op = nc.declare_dram_parameter("offs", [tw, nbatch * dcols], F32, False)
    ex = nc.declare_dram_parameter(
        "expected", [tw, nbatch * dcols], F32, isOutput=True
    )

    nbuf = min(ngrp, 4)
    gdc = ntile * 24            # e-columns per group
    from contextlib import ExitStack

    with ExitStack() as st:
        ec = st.enter_context
        tlg = ec(nc.sbuf_tensor([P, nbuf, P1_GRP * cc], F16))
        te = ec(nc.sbuf_tensor([P, nbuf, P1_GRP * cc], F16))
        tw_ = ec(nc.sbuf_tensor([P, 32], F16))
        tid = ec(nc.sbuf_tensor([P, 96], F16))
        twu = ec(nc.sbuf_tensor([P, 256], F16))        # PE warmup source
        sbc = ec(nc.sbuf_tensor([P, 2, cc], F16))      # psum evacuation
        trec = ec(nc.sbuf_tensor([P, gdc], F32))
        toffs = ec(nc.sbuf_tensor([P, ngrp * gdc], F32))
        ebuf = ec(nc.sbuf_tensor([P, ngrp * gdc], F32))
        ps = [
            ec(nc.psum_tensor(f"ps{i}", [P, cc], F32)) for i in range(2)
        ]
        pt = [
            ec(nc.psum_tensor(f"pt{i}", [P, ntile, 96], F16))
            for i in range(2)
        ]
        pwu = ec(nc.psum_tensor("pwu", [32, 256], F32))
        block = ec(nc.Block())
        wsem = ec(nc.semaphore("wsem"))
        usem = ec(nc.semaphore("usem"))
        lsem = ec(nc.semaphore("lsem"))
        esem = ec(nc.semaphore("esem"))
        msem = ec(nc.semaphore("msem"))
        csem = ec(nc.semaphore("csem"))
        tsem = ec(nc.semaphore("tsem"))
        vsem = ec(nc.semaphore("vsem"))
        osem = ec(nc.semaphore("osem"))

        @block.sync
        def _(sync):
            for gi in range(ngrp):
                if gi >= nbuf:
                    sync.wait_ge(esem, gi - nbuf + 1)    # tlg buffer reuse
                c0 = cum[gi] - gs[gi]
                sync.dma_start(
                    out=tlg[:, gi % nbuf, 0 : gs[gi] * cc],
                    in_=lg[:, c0 * cc : cum[gi] * cc],
                ).then_inc(lsem, 16)
            half_b = nbatch // 2
            sync.wait_ge(vsem, 2 * half_b)
            sync.dma_start(
                out=ex[:, 0 : 2 * half_b * gdc],
                in_=ebuf[0:tw, 0 : 2 * half_b * gdc],
            ).then_inc(osem, 16)
            sync.wait_ge(vsem, ngrp)
            sync.dma_start(
                out=ex[:, 2 * half_b * gdc :],
                in_=ebuf[0:tw, 2 * half_b * gdc :],
            ).then_inc(osem, 16)
            sync.wait_ge(osem, 32)

        @block.scalar
        def _(scalar):
            scalar.dma_start(out=tw_[:], in_=wp[:]).then_inc(wsem, 16)
            scalar.dma_start(out=tid[:], in_=idp[:]).then_inc(wsem, 16)
            scalar.dma_start(out=toffs[0:tw, :], in_=op[:]).then_inc(wsem, 16)
            for gi in range(ngrp):
                scalar.wait_ge(lsem, 16 * (gi + 1))
                if gi >= nbuf:
                    scalar.wait_ge(msem, cum[gi - nbuf])   # te buffer reuse
                nc.scalar.activation(
                    out=te[:, gi % nbuf, 0 : gs[gi] * cc],
                    in_=tlg[:, gi % nbuf, 0 : gs[gi] * cc],
                    func=mybir.ActivationFunctionType.Exp,
                ).then_inc(esem, 1)

        @block.gpsimd
        def _(gpsimd):
            gpsimd.memset(twu[:], 0.0)
            gpsimd.drain().then_inc(usem, 1)
            for gi in range(ngrp):
                gpsimd.wait_ge(msem, cum[gi])
                if gi >= 2:
                    gpsimd.wait_ge(tsem, ntile * (gi - 1))   # sbc reuse
                nc.gpsimd.tensor_copy(
                    out=sbc[0:96, gi % 2, :], in_=ps[gi % 2][0:96, :]
                ).then_inc(csem, 1)

        def _mm_group(tensor, gi):
            tensor.wait_ge(esem, gi + 1)
            if gi >= 2:
                tensor.wait_ge(csem, gi - 1)   # psum tile reuse
            for t in range(gs[gi]):
                nc.tensor.matmul(
                    out=ps[gi % 2][32 * t : 32 * t + 32, :],
                    lhsT=tw_[:],
                    rhs=te[:, gi % nbuf, t * cc : (t + 1) * cc],
                    start=True, stop=True,
                ).then_inc(msem, 1)

        def _tr_group(tensor, gi):
            tensor.wait_ge(csem, gi + 1)
            if gi >= 2:
                tensor.wait_ge(vsem, gi - 1)   # psumT tile reuse
            for k in range(ntile):
                nc.tensor.transpose(
                    out=pt[gi % 2][0:tw, k, :],
                    in_=sbc[0:96, gi % 2, k * tw : (k + 1) * tw],
                    identity=tid[0:96, :],
                ).then_inc(tsem, 1)

        @block.tensor
        def _(tensor):
            # keep the PE busy early so it is at full p-state for real work
            tensor.wait_ge(usem, 1)
            for _ in range(14):
                nc.tensor.matmul(
                    out=pwu[:, :], lhsT=twu[:, 0:32], rhs=twu[:],
                    start=True, stop=True,
                )
            tensor.wait_ge(wsem, 32)
            _mm_group(tensor, 0)
            for gi in range(1, ngrp):
                _mm_group(tensor, gi)
                _tr_group(tensor, gi - 1)
            _tr_group(tensor, ngrp - 1)

        def _div(vector, gi):
            vector.wait_ge(tsem, ntile * (gi + 1))
            pb = pt[gi % 2][0:tw, :, :].rearrange("p a (t m) -> p a t m", m=32)
            sl = slice(gi * gdc, (gi + 1) * gdc)
            nc.vector.reciprocal(out=trec[0:tw, :], in_=pb[:, :, :, 0:8])
            nc.vector.tensor_tensor(
                out=ebuf[0:tw, sl], in0=pb[:, :, :, 8:16],
                in1=trec[0:tw, :], op=mybir.AluOpType.mult,
            )
            nc.vector.tensor_tensor(
                out=ebuf[0:tw, sl], in0=ebuf[0:tw, sl],
                in1=toffs[0:tw, sl], op=mybir.AluOpType.add,
            ).then_inc(vsem, 1)

        @block.vector
        def _(vector):
            vector.wait_ge(wsem, 48)
            for gi in range(ngrp):
                _div(vector, gi)

    return nc


# --------------------------------------------------------------------------
# phase 2: w = xg*ft, tree segment-sum -> Ax, viol stats
# --------------------------------------------------------------------------
def _build_phase2(tiers):
    """tiers: tuple of (L, Spad, Rc) per degree tier; chunk = Rc ranks.

    Per chunk: DVE multiplies xg*ft in place, GPSIMD does the big first
    halving of the slot tree, DVE finishes the tree into tax.  Stats run
    in two pieces (mid-stream + tail) and are combined."""
    global LAST_P2_ARGS
    LAST_P2_ARGS = (tiers,)
    nc = bass.Bass()
    ax_tot = sum(s for _, s, _ in tiers)
    chunks = []          # (tier_idx, chunk_idx, axbase)
    axb = 0
    for t, (L, Spad, Rc) in enumerate(tiers):
        for ch in range(Spad // Rc):
            chunks.append((t, ch, axb + ch * Rc))
        axb += Spad
    chmax = max(2 * L * Rc for L, _, Rc in tiers)
    nchunks = len(chunks)
    # stats split points: pieces end at tier boundaries crossing ~50%/85%
    assert len(tiers) >= 2
    bounds = []          # (after_chunk_idx, lo, hi) per piece
    lo = 0
    acc = 0
    thr = [0.5, 0.85]
    for t, (L, Spad, Rc) in enumerate(tiers[:-1]):
        acc += Spad
        if len(bounds) < 2 and acc >= thr[len(bounds)] * ax_tot:
            cidx = sum(s // r for _, s, r in tiers[: t + 1]) - 1
            bounds.append((cidx, lo, acc))
            lo = acc
    bounds.append((nchunks - 1, lo, ax_tot))
    npieces = len(bounds)

    xs = [
        nc.declare_dram_parameter(f"st{t}", [P, Spad * 2 * L], F16, False)
        for t, (L, Spad, Rc) in enumerate(tiers)
    ]
    bs = nc.declare_dram_parameter("bias", [P, ax_tot], F16, isOutput=False)
    out_p = nc.declare_dram_parameter("partials", [P, 4], F32, isOutput=True)

    with (
        nc.sbuf_tensor([P, NBUF2, chmax], F16) as tst,
        nc.sbuf_tensor([P, ax_tot], F16) as tax,
        nc.sbuf_tensor([P, ax_tot], F16) as tb,
        nc.sbuf_tensor([P, ax_tot], F16) as tv,
        nc.sbuf_tensor([P, npieces], F16) as tm16,
        nc.sbuf_tensor([P, npieces], F32) as tsum,
        nc.sbuf_tensor([P, npieces], F32) as tcnt,
        nc.sbuf_tensor([P, 4], F32) as tout,
        nc.Block() as block,
        nc.semaphore("bsem") as bsem,
        nc.semaphore("pa") as pa,
        nc.semaphore("m2") as m2,
        nc.semaphore("g2") as g2,
        nc.semaphore("vs") as vs,
        nc.semaphore("fsem") as fsem,
        nc.semaphore("osem") as osem,
    ):

        def _l1_split(i):
            t, ch, axb_c = chunks[i]
            L, Spad, Rc = tiers[t]
            h1 = L if L % 2 == 0 else L - 1
            hf = (h1 // 2) * Rc
            xc = min(hf, (int(0.17 * hf) + 7) // 8 * 8)
            return hf, xc

        @block.sync
        def _(sync):
            for i, (t, ch, _axb) in enumerate(chunks):
                L, Spad, Rc = tiers[t]
                sz = 2 * L * Rc
                if i >= NBUF2:
                    sync.wait_ge(vs, i - NBUF2 + 1)
                sync.dma_start(
                    out=tst[:, i % NBUF2, 0:sz],
                    in_=xs[t][:, ch * sz : (ch + 1) * sz],
                ).then_inc(pa, 16)
            sync.wait_ge(fsem, 2)
            sync.dma_start(out=out_p[:], in_=tout[:]).then_inc(osem, 16)
            sync.wait_ge(osem, 16)

        @block.scalar
        def _(scalar):
            scalar.dma_start(out=tb[:], in_=bs[:]).then_inc(bsem, 16)

        @block.gpsimd
        def _(gpsimd):
            for i, (t, ch, axb_c) in enumerate(chunks):
                hf, xc = _l1_split(i)
                gpsimd.wait_ge(m2, i + 1)
                w = tst[:, i % NBUF2, :]
                if xc < hf:
                    nc.gpsimd.tensor_tensor(
                        out=w[0:P, xc:hf], in0=w[0:P, xc:hf],
                        in1=w[0:P, hf + xc : 2 * hf], op=mybir.AluOpType.add,
                    ).then_inc(g2, 1)
                else:
                    gpsimd.sem_inc(g2, 1)

        def _tree_upper(i):
            # assumes the full first halving (DVE part + GPSIMD part) is done
            t, ch, axb_c = chunks[i]
            L, Spad, Rc = tiers[t]
            w = tst[:, i % NBUF2, :]
            if L % 2 == 1:
                nc.vector.tensor_tensor(
                    out=w[0:P, 0:Rc], in0=w[0:P, 0:Rc],
                    in1=w[0:P, (L - 1) * Rc : L * Rc],
                    op=mybir.AluOpType.add,
                )
            h = (L if L % 2 == 0 else L - 1) // 2
            if h <= 1:
                nc.vector.tensor_copy(
                    out=tax[:, axb_c : axb_c + Rc], in_=w[0:P, 0:Rc]
                ).then_inc(vs, 1)
                return
            while True:
                if h % 2 == 1:
                    nc.vector.tensor_tensor(
                        out=w[0:P, 0:Rc], in0=w[0:P, 0:Rc],
                        in1=w[0:P, (h - 1) * Rc : h * Rc],
                        op=mybir.AluOpType.add,
                    )
                    h -= 1
                if h == 2:
                    nc.vector.tensor_tensor(
                        out=tax[:, axb_c : axb_c + Rc],
                        in0=w[0:P, 0:Rc], in1=w[0:P, Rc : 2 * Rc],
                        op=mybir.AluOpType.add,
                    ).then_inc(vs, 1)
                    break
                hf = (h // 2) * Rc
                nc.vector.tensor_tensor(
                    out=w[0:P, 0:hf], in0=w[0:P, 0:hf],
                    in1=w[0:P, hf : 2 * hf],
                    op=mybir.AluOpType.add,
                )
                h //= 2

        def _stats(piece, lo, hi):
            nc.vector.tensor_tensor(
                out=tv[:, lo:hi], in0=tax[:, lo:hi], in1=tb[:, lo:hi],
                op=mybir.AluOpType.subtract,
            )
            nc.vector.tensor_scalar_max(
                out=tv[:, lo:hi], in0=tv[:, lo:hi], scalar1=0.0
            )
            nc.vector.tensor_reduce(
                out=tsum[:, piece : piece + 1], in_=tv[:, lo:hi],
                axis=mybir.AxisListType.X, op=mybir.AluOpType.add,
            )
            nc.vector.tensor_reduce(
                out=tm16[:, piece : piece + 1], in_=tv[:, lo:hi],
                axis=mybir.AxisListType.X, op=mybir.AluOpType.max,
            )
            nc.vector.tensor_scalar(
                out=tv[:, lo:hi], in0=tv[:, lo:hi], scalar1=CNT_THR,
                scalar2=None, op0=mybir.AluOpType.is_gt,
            )
            nc.vector.tensor_reduce(
                out=tcnt[:, piece : piece + 1], in_=tv[:, lo:hi],
                axis=mybir.AxisListType.X, op=mybir.AluOpType.add,
            )

        @block.vector
        def _(vector):
            for i, (t, ch, axb_c) in enumerate(chunks):
                L, Spad, Rc = tiers[t]
                m = L * Rc
                vector.wait_ge(pa, 16 * (i + 1))
                w = tst[:, i % NBUF2, :]
                nc.vector.tensor_tensor(
                    out=w[0:P, 0:m], in0=w[0:P, 0:m], in1=w[0:P, m : 2 * m],
                    op=mybir.AluOpType.mult,
                ).then_inc(m2, 1)
                hf, xc = _l1_split(i)
                if xc > 0:
                    nc.vector.tensor_tensor(
                        out=w[0:P, 0:xc], in0=w[0:P, 0:xc],
                        in1=w[0:P, hf : hf + xc], op=mybir.AluOpType.add,
                    )
                if i > 0:
                    vector.wait_ge(g2, i)
                    _tree_upper(i - 1)
                for pi, (cidx, plo, phi) in enumerate(bounds):
                    if i - 1 == cidx and cidx < nchunks - 1:
                        vector.wait_ge(bsem, 16)
                        _stats(pi, plo, phi)
            vector.wait_ge(g2, nchunks)
            _tree_upper(nchunks - 1)
            vector.wait_ge(bsem, 16)
            for pi, (cidx, plo, phi) in enumerate(bounds):
                if cidx == nchunks - 1:
                    _stats(pi, plo, phi)
            # combine the pieces
            nc.vector.tensor_reduce(
                out=tout[:, 0:1], in_=tsum[:],
                axis=mybir.AxisListType.X, op=mybir.AluOpType.add,
            )
            nc.vector.tensor_reduce(
                out=tout[:, 1:2], in_=tm16[:],
                axis=mybir.AxisListType.X, op=mybir.AluOpType.max,
            )
            nc.vector.tensor_reduce(
                out=tout[:, 2:3], in_=tcnt[:],
                axis=mybir.AxisListType.X, op=mybir.AluOpType.add,
            )
            nc.vector.tensor_copy(out=tout[:, 3:4], in_=tout[:, 2:3]).then_inc(
                fsem, 2
            )

    return nc


def _ceil_div(a, b):
    return -(-a // b)


# --------------------------------------------------------------------------
# host-side layout prep (index shuffling only)
# --------------------------------------------------------------------------
def _p1_rowmap(nch, cc):
    """row_of[j, bi, ti, t, g] -> packed row index (or -1 if the slot is
    junk), matching the device's e/offs column order col = bi*dcols +
    ti*24 + t*8 + g with partition j."""
    gs = _p1_groups(nch)
    ngrp = len(gs)
    nbatch = _ceil_div(ngrp, 2)
    tw = P1_TW
    ntile = cc // tw
    ncol = nch * cc
    j, bi, ti, t, g = np.meshgrid(
        np.arange(tw), np.arange(nbatch), np.arange(2 * ntile),
        np.arange(P1_GRP), np.arange(8), indexing="ij",
    )
    gi = bi * 2 + ti // ntile
    k4 = ti % ntile
    valid = (gi < ngrp) & (t < np.asarray(gs + [0])[np.minimum(gi, ngrp)])
    chunk = np.cumsum([0] + gs)[np.minimum(gi, ngrp - 1)] + t
    row = g * ncol + chunk * cc + k4 * tw + j
    row = np.where(valid, row, -1)
    return row, ncol


def _prep_phase1(logits, offsets):
    """Pack per-core class-major logits grids + offsets; return arrays."""
    ns, ccls = logits.shape
    assert ccls == 16
    rows_core = _ceil_div(ns, NCORES)
    ncol_need = _ceil_div(rows_core, 8)
    cc = P1_CC
    nch = _ceil_div(ncol_need, cc)
    ncol = nch * cc
    rows_cap = 8 * ncol
    tw = P1_TW

    # weight block: cols 0..7 ones per group, 8..15 class values, 16..31 zero
    W = np.zeros((P, 32), dtype=np.float16)
    pidx = np.arange(P)
    g = pidx // 16
    c = pidx % 16
    W[pidx, g] = 1.0
    W[pidx, 8 + g] = c.astype(np.float16)
    ident = np.zeros((P, 96), dtype=np.float16)
    ident[np.arange(96), np.arange(96)] = 1.0

    row_of, _ = _p1_rowmap(nch, cc)
    flat = row_of.reshape(tw, -1)

    lgs, offs_packed = [], []
    for core in range(NCORES):
        lo, hi = core * rows_core, min((core + 1) * rows_core, ns)
        lgp = np.zeros((rows_cap, ccls), dtype=np.float16)
        lgp[: hi - lo] = logits[lo:hi].astype(np.float16)
        # partition p = g*16 + cls, column j; row r = g*ncol + j
        lgs.append(
            np.ascontiguousarray(
                lgp.reshape(8, ncol, ccls).transpose(0, 2, 1).reshape(P, ncol)
            )
        )
        ofp = np.zeros(rows_cap + 1, dtype=np.float32)
        ofp[: hi - lo] = offsets[lo:hi]
        offs_packed.append(np.ascontiguousarray(ofp[flat]))
    return (nch, cc), W, ident, lgs, offs_packed, rows_core, rows_cap


def _unpack_expected(e_packed, nch, cc, rows_cap, nrows):
    row_of, _ = _p1_rowmap(nch, cc)
    flat = row_of.reshape(-1)
    ok = flat >= 0
    out = np.zeros(rows_cap, dtype=np.float32)
    out[flat[ok]] = e_packed.reshape(-1)[ok]
    return out[:nrows]


def _prep_phase2(con, var, feat, bias, n_con):
    """Sort edges, tier by degree, slot-major layout. Returns metadata +
    per-tier (core-major) index/feature arrays; xg filled later."""
    ne = con.shape[0]
    deg = np.bincount(con, minlength=n_con)
    order = np.argsort(con, kind="stable")
    run_start = np.zeros(n_con + 1, dtype=np.int64)
    np.cumsum(deg, out=run_start[1:])
    con_sorted = con[order]
    off_in_run = np.arange(ne, dtype=np.int64) - run_start[con_sorted]
    var_sorted = var[order]
    feat_sorted = feat[order]

    maxdeg = int(deg.max()) if ne else 1
    cand = list(range(16, 68, 4))
    if maxdeg > cand[-1]:
        cand.append(_ceil_div(maxdeg, 4) * 4)
    cand = np.asarray(cand, dtype=np.int64)
    t_cand = np.searchsorted(cand, deg, side="left")
    cnt = np.bincount(t_cand, minlength=len(cand))
    # merge small tiers upward into the next stride
    keep = []
    acc = 0
    remap = np.zeros(len(cand), dtype=np.int64)
    for si in range(len(cand)):
        acc += cnt[si]
        remap[si] = len(keep)
        if (acc >= MIN_TIER) or (si == len(cand) - 1 and acc > 0):
            keep.append(int(cand[si]))
            acc = 0
    t_of_seg = remap[t_cand]

    raw = []
    for t, L in enumerate(keep):
        segs = np.nonzero(t_of_seg == t)[0]
        n_t = segs.shape[0]
        if n_t == 0:
            continue
        S_t = _ceil_div(n_t, NBINS)
        nch_t = max(1, int(round(S_t * 2 * L / CH_TARGET)))
        nch_t = min(nch_t, S_t)
        Rc = _ceil_div(S_t, nch_t)
        Spad = nch_t * Rc
        raw.append((t, L, Spad, Rc, segs))
    # processing order = tier order: small tier first and last (short
    # pipeline fill/drain), big tiers in the middle
    order = sorted(range(len(raw)), key=lambda i: -raw[i][2] * raw[i][1])
    if len(order) >= 3:
        order = [order[-2]] + order[:-2] + [order[-1]]

    tiers = []
    tier_data = []
    axb = 0
    dense = np.full(len(keep), -1, dtype=np.int64)
    for i in order:
        t, L, Spad, Rc, segs = raw[i]
        dense[t] = len(tiers)
        k_of_con = np.full(n_con, -1, dtype=np.int64)
        k_of_con[segs] = np.arange(segs.shape[0])
        tiers.append((L, Spad, Rc))
        tier_data.append((segs, k_of_con, axb))
        axb += Spad
    t_of_seg = dense[t_of_seg]

    ax_tot = axb
    bias_arr = np.full((NCORES, P, ax_tot), BIG_BIAS, dtype=np.float16)
    for (L, Spad, Rc), (segs, k_of_con, axb) in zip(tiers, tier_data):
        k = k_of_con[segs]
        bb = k % NBINS
        r = k // NBINS
        bias_arr[bb // P, bb % P, axb + r] = bias[segs].astype(np.float16)

    return (
        tiers,
        tier_data,
        ax_tot,
        bias_arr,
        con_sorted,
        off_in_run,
        var_sorted,
        feat_sorted,
        t_of_seg,
    )


def _fill_streams(tiers, tier_data, t_of_seg, con_sorted, off_in_run,
                  var_sorted, feat_sorted, x16):
    """Build per-tier interleaved (xg, ft) fp16 streams, slot-major."""
    e_tier = t_of_seg[con_sorted]
    streams = []
    for t, ((L, Spad, Rc), (segs, k_of_con, axb)) in enumerate(
        zip(tiers, tier_data)
    ):
        sel = np.nonzero(e_tier == t)[0]
        cs = con_sorted[sel]
        slot = off_in_run[sel]
        k = k_of_con[cs]
        b = k % NBINS
        r = k // NBINS
        core = b // P
        part = b % P
        ch = r // Rc
        rin = r % Rc
        base = ch * (2 * L * Rc)
        col_x = base + slot * Rc + rin
        col_f = base + (L + slot) * Rc + rin
        width = Spad * 2 * L
        arr = np.zeros(NCORES * P * width, dtype=np.float16)
        flat_base = (core * P + part) * width
        arr[flat_base + col_x] = x16[var_sorted[sel]]
        arr[flat_base + col_f] = feat_sorted[sel].astype(np.float16)
        streams.append(arr.reshape(NCORES, P, width))
    return streams


# --------------------------------------------------------------------------
def kernel(**inputs) -> tuple:
    prob_bin = np.asarray(inputs["prob_bin"], dtype=np.float32)
    logits = np.asarray(inputs["logits_int_small"], dtype=np.float32)
    offsets = np.asarray(inputs["int_small_offsets"], dtype=np.float32)
    pred_l = np.asarray(inputs["pred_int_large"], dtype=np.float32)
    feat = np.asarray(inputs["edge_features"], dtype=np.float32).reshape(-1)
    cfeat = np.asarray(inputs["constraint_features"], dtype=np.float32)
    vfeat = np.asarray(inputs["variable_features"], dtype=np.float32)
    idx_bin = np.asarray(inputs["idx_bin"], dtype=np.int64)
    idx_s = np.asarray(inputs["idx_int_small"], dtype=np.int64)
    idx_l = np.asarray(inputs["idx_int_large"], dtype=np.int64)
    var_types = np.asarray(inputs["var_types"], dtype=np.int64)
    ei = np.asarray(inputs["edge_indices"], dtype=np.int64)
    n_vars = int(inputs["n_vars"])

    n_con = cfeat.shape[0]
    ns = logits.shape[0]
    bias = np.ascontiguousarray(cfeat[:, BIAS_COL])
    lp_vals = np.ascontiguousarray(vfeat[:, LP_SOL_COL])

    # ---------------- launch 1 ----------------
    (nch, cc), W, ident, lgs, offs_packed, rows_core, rows_cap = _prep_phase1(
        logits, offsets
    )
    nc1 = _build_phase1((nch, cc))
    in1 = [
        {"logits": lgs[c], "wmat": W, "ident": ident, "offs": offs_packed[c]}
        for c in range(NCORES)
    ]
    res1 = run_bass_kernel_spmd(nc1, in1, list(range(NCORES)))
    expected = np.concatenate(
        [
            _unpack_expected(
                res1.results[c]["expected"], nch, cc, rows_cap, rows_core
            )
            for c in range(NCORES)
        ]
    )[:ns]

    # ---------------- host: assemble x ----------------
    xfull = np.zeros(n_vars, dtype=np.float32)
    xfull[idx_bin] = prob_bin[:, 0]
    xfull[idx_s] = expected
    xfull[idx_l] = pred_l[:, 0]
    xfull = np.where(var_types == 0, lp_vals, xfull)
    x16 = xfull.astype(np.float16)

    # ---------------- launch 2 ----------------
    (
        tiers, tier_data, ax_tot, bias_arr, con_sorted, off_in_run,
        var_sorted, feat_sorted, t_of_seg,
    ) = _prep_phase2(ei[0], ei[1], feat, bias, n_con)
    streams = _fill_streams(
        tiers, tier_data, t_of_seg, con_sorted, off_in_run, var_sorted,
        feat_sorted, x16,
    )
    nc2 = _build_phase2(tuple(tiers))
    in2 = []
    for c in range(NCORES):
        m = {f"st{t}": streams[t][c] for t in range(len(tiers))}
        m["bias"] = bias_arr[c]
        in2.append(m)
    res2 = run_bass_kernel_spmd(nc2, in2, list(range(NCORES)))

    parts = np.stack([res2.results[c]["partials"] for c in range(NCORES)])
    vsum = parts[:, :, 0].astype(np.float64).sum()
    vmax = np.float32(parts[:, :, 1].max())
    vcnt = np.int64(round(float(parts[:, :, 2].astype(np.float64).sum())))
    mean_viol = np.float32(vsum / np.float64(n_con))
    penalty = np.float32(
        np.float32(LAMBDA_MEAN) * mean_viol + np.float32(LAMBDA_MAX) * vmax
    )
    return penalty, mean_viol, vmax, vcnt
